# revision 35
# baseline (speedup 1.0000x reference)
"""Trainium2 Bass kernel for nn_Attention_4_lora (B=8, T=1024, C=1024, R=64).

Strategy: data-parallel over the batch dim (1 batch per NeuronCore, 8 cores).
All activations live in transposed [channel, token] layout so that every
matmul contraction runs over the SBUF partition axis. BatchNorm statistics
are reduced across cores with two small AllReduces. All matmul operands are
bf16 (full PE rate, FWL fast-weight-load, half the SBUF/DMA traffic of
f32r); accumulation stays fp32 in PSUM.

Per-core pipeline:
  P1  merge Wm_attn^T = W_attn^T + reshape(A@B)^T on device, in d-HALVES.
      The K=64 LoRA delta matmuls run PAIRED via 2x row tiling (operands
      duplicated on partitions 0-63 / 64-127), so two deltas share the PE
      array. The strided adds alternate between Vector and GpSimd.
  WEFF (between the P2 halves) this core's 128-row shard of
      W_eff^T = Wp^T @ Wmp^T  (Wmp = Wp + lpA@lpB), via
      Z = lpB @ Wp[:, shard]  then  shard = Wp[:, shard]^T Wp^T + Z^T lpA^T
      -- all chained N=512 matmuls; bf16 AllGather of the 8 shards ->
      every core holds the full [C, C] W_eff^T. The readback DMAs are
      issued right after the qk AllReduce so the 2MB (1024-descriptor)
      transfer drains during P3, far away from the P5/P6 seam.
  P2  xa^T[d, t] = Wm^T-slab.T @ x^T for q (half 0) and k (half 1),
      bn_stats per tile
  P3  v[t, c] (natural layout, needed as AV stationary); v stats are
      accumulated across token tiles on GpSimd (sum v) and Vector (sum v^2)
      in f32r, then reduced over the partition axis with 4 ones-matmuls;
      the v AllReduce fires at P3 end.
  P5  scores^T[s, t] in 256-token chunks (finer causal skipping),
      exp((q.k)/32) on ScalarE, causal mask via affine_select, row-sums
      via ones-matmul into half-banks; 1/r via a [128,4] reshape bounce
      (DVE reciprocal is free-dim paced).
  P6  y^T[c, t] = v-slab.T @ att_exp^T; drain fuses 1/r (Vector) and the
      BN-v scale/bias y = y*va[c] + vb[c] (GpSimd), so P7 needs no bias.
  P7  single projection: out^T = W_eff^T-slab.T @ y^T -> [C, T] in bf16.

kernel() takes the full unsharded inputs, shards/uploads (weights cast to
bf16 on host), runs SPMD on cores 0-7, gathers, and transposes back to
[B, T, C].
"""

import ml_dtypes
import numpy as np

import concourse.bass as bass
import concourse.mybir as mybir
import concourse.tile as tile
from concourse import bacc
from concourse.bass_utils import run_bass_kernel_spmd

NCORES = 8
C = 1024
R = 64
D3 = 3 * C
EPS = 1e-5
F32 = mybir.dt.float32
F32R = mybir.dt.float32r
BF16 = mybir.dt.bfloat16
F8 = mybir.dt.float8e4
FP8_P7 = False     # fp8e4 DoubleRow P7 measured 3.6e-2 rel err (> 2e-2 gate)
WSCALE = 1024.0    # W_eff prescale into the fp8e4 normal range
AX = mybir.AxisListType
OP = mybir.AluOpType
ACTF = mybir.ActivationFunctionType
BF16NP = ml_dtypes.bfloat16


def _erange(f, d0, d1):
    """e-range such that d = 3e + f lies in [d0, d1)."""
    el = -((-(d0 - f)) // 3)
    eh = -((-(d1 - f)) // 3)
    return el, eh


def build(T=1024, single_core=False, no_collective=False, reps=1):
    NT = T // 128          # 128-token tiles
    TQ = T // 512          # 512-token chunks
    assert T % 512 == 0

    nc = bacc.Bacc(None, target_bir_lowering=False,
                   num_devices=(1 if single_core else NCORES))

    prm = {}
    prm["xT"] = nc.declare_dram_parameter("xT", [C, T], BF16, isOutput=False)
    prm["wT"] = nc.declare_dram_parameter("wT", [C, D3], BF16, isOutput=False)
    prm["wpT"] = nc.declare_dram_parameter("wpT", [C, C], BF16, isOutput=False)
    prm["wpN"] = nc.declare_dram_parameter("wpN", [C, 128], BF16, isOutput=False)
    prm["laT"] = nc.declare_dram_parameter("laT", [R, C], BF16, isOutput=False)
    prm["lbB"] = nc.declare_dram_parameter("lbB", [R, D3], BF16, isOutput=False)
    prm["lpaT"] = nc.declare_dram_parameter("lpaT", [R, C], BF16, isOutput=False)
    prm["lpbN"] = nc.declare_dram_parameter("lpbN", [C, R], BF16, isOutput=False)
    prm["gam"] = nc.declare_dram_parameter("gam", [D3], F32, isOutput=False)
    prm["bet"] = nc.declare_dram_parameter("bet", [D3], F32, isOutput=False)
    prm["out"] = nc.declare_dram_parameter("out", [C, T], BF16, isOutput=True)

    with tile.TileContext(nc) as tc:
        for rep in range(reps):
            _emit(nc, tc, prm, T, rep, single_core, no_collective)

    nc.compile()
    return nc


def _emit(nc, tc, prm, T, rep, single_core, no_collective):
    NT = T // 128
    TQ = T // 512
    TC4 = T // 256
    xT, wT, wpT, wpN = prm["xT"], prm["wT"], prm["wpT"], prm["wpN"]
    laT, lbB = prm["laT"], prm["lbB"]
    lpaT, lpbN, gam, bet, out = prm["lpaT"], prm["lpbN"], prm["gam"], prm["bet"], prm["out"]

    stats_in = nc.dram_tensor(f"stats_in_{rep}", [4096], F32)
    stats_out = nc.dram_tensor(f"stats_out_{rep}", [4096], F32)
    vstats_in = nc.dram_tensor(f"vstats_in_{rep}", [2 * C], F32)
    vstats_out = nc.dram_tensor(f"vstats_out_{rep}", [2 * C], F32)
    rb_dram = nc.dram_tensor(f"rb_{rep}", [T], F32)
    rb2_dram = nc.dram_tensor(f"rb2_{rep}", [T], F32)
    wdt = F8 if FP8_P7 else BF16
    weff_in = nc.dram_tensor(f"weff_in_{rep}", [128 * C], wdt)
    weff_out = nc.dram_tensor(f"weff_out_{rep}", [C * C], wdt,
                              addr_space="Shared")

    def bcast_dram(param, offset, n):
        return bass.AP(tensor=param[:].tensor, offset=offset, ap=[[0, 128], [1, n]])

    def all_reduce(ins, outs):
        if single_core or no_collective:
            nc.sync.dma_start(out=outs, in_=ins)
        else:
            nc.gpsimd.collective_compute(
                "AllReduce", OP.add,
                replica_groups=[list(range(NCORES))],
                ins=[ins], outs=[outs])

    with (
        tc.tile_pool(name=f"misc{rep}", bufs=1) as misc,
        tc.tile_pool(name=f"outst{rep}", bufs=2) as outst,
        tc.tile_pool(name=f"vpool{rep}", bufs=1) as vpool,
        tc.tile_pool(name=f"attp{rep}", bufs=1) as attp,
        tc.tile_pool(name=f"psA{rep}", bufs=4, space="PSUM") as psA,
    ):
        # ---------------- constants / small loads ----------------
        ones_f = misc.tile([128, 1], F32)
        nc.vector.memset(ones_f[:, :], 1.0)
        ones_b = misc.tile([128, 1], BF16)
        nc.vector.tensor_copy(out=ones_b[:, :], in_=ones_f[:, :])
        ones_r = misc.tile([128, 1], F32R)
        nc.vector.tensor_copy(out=ones_r[:, :], in_=ones_f[:, :])
        eps_t = misc.tile([128, 1], F32)
        nc.vector.memset(eps_t[:, :], EPS)

        qk_mv = misc.tile([128, 16, 2], F32)
        m16 = misc.tile([128, 16], F32)
        qa = misc.tile([128, 16], F32)
        qb = misc.tile([128, 16], F32)

        xa = [None] * 16
        vnat = [None] * NT
        weff = [None] * 8

        with tc.tile_pool(name=f"xapool{rep}", bufs=1) as xapool:
          with tc.tile_pool(name=f"projp{rep}", bufs=1) as projp:
            with tc.tile_pool(name=f"lorap{rep}", bufs=1) as lorap:
                # la/lb duplicated on partitions 0-63 and 64-127 so the K=64
                # delta matmuls can run pairwise via 2x row tiling; the
                # second copy is an SBUF->SBUF DMA (no HBM bandwidth)
                la2 = lorap.tile([128, C], BF16)
                nc.sync.dma_start(out=la2[0:R, :], in_=laT[:, :])
                nc.sync.dma_start(out=la2[R:2 * R, :], in_=la2[0:R, :])
                lb2 = lorap.tile([128, D3], BF16)
                for _c in range(3):
                    nc.sync.dma_start(out=lb2[0:R, 1024 * _c:1024 * (_c + 1)],
                                      in_=lbB[:, 1024 * _c:1024 * (_c + 1)])
                    nc.sync.dma_start(out=lb2[R:2 * R, 1024 * _c:1024 * (_c + 1)],
                                      in_=lb2[0:R, 1024 * _c:1024 * (_c + 1)])

                with tc.tile_pool(name=f"xtpool{rep}", bufs=1) as xtpool:
                    with tc.tile_pool(name=f"wb{rep}", bufs=1) as wbp:
                        # -------- P1 merge (d-halves, paired deltas) ------
                        def merge_half(d0, interleave=None):
                            """Merged Wm^T[:, d0:d0+1024] as 8 c-tiles [128, 1032]."""
                            wq = []
                            for ct in range(8):
                                w_t = wbp.tile([128, 1032], BF16, tag=f"wb{ct}",
                                               bufs=(2 if ct < 4 else 1),
                                               name=f"wh{d0}_{ct}")
                                nc.sync.dma_start(
                                    out=w_t[:, 0:1024],
                                    in_=wT[128 * ct:128 * (ct + 1), d0:d0 + 1024])
                                if interleave is not None:
                                    interleave(ct)
                                wq.append(w_t)
                            jobs = [(ct, f) for ct in range(8) for f in range(3)]
                            for j0 in range(0, len(jobs), 2):
                                pss = []
                                for pi, (ct, f) in enumerate(jobs[j0:j0 + 2]):
                                    r0 = R * pi      # partition offset 0 / 64
                                    el, eh = _erange(f, d0, d0 + 1024)
                                    cnt = eh - el
                                    ps = psA.tile([128, 512], F32, tag="mm",
                                                  name=f"dps{d0}_{ct}_{f}")
                                    nc.tensor.matmul(
                                        ps[:, 0:cnt],
                                        lb2[r0:r0 + R,
                                            1024 * f + 128 * ct:1024 * f + 128 * (ct + 1)],
                                        la2[r0:r0 + R, el:el + cnt],
                                        start=True, stop=True)
                                    pss.append(ps)
                                for pi, (ct, f) in enumerate(jobs[j0:j0 + 2]):
                                    w_t = wq[ct]
                                    view3 = w_t[:, :].rearrange(
                                        "p (u three) -> p u three", three=3)
                                    el, eh = _erange(f, d0, d0 + 1024)
                                    cnt = eh - el
                                    c0 = 3 * el + f - d0
                                    ps = pss[pi]
                                    if (ct + f) % 2 == 0:
                                        nc.vector.tensor_tensor(
                                            out=view3[:, 0:cnt, c0],
                                            in0=view3[:, 0:cnt, c0],
                                            in1=ps[:, 0:cnt], op=OP.add)
                                    else:
                                        # GpSimd cannot read PSUM: stage via
                                        # Scalar, add on GpSimd
                                        stg = misc.tile([128, 344], BF16,
                                                        tag="mgst", bufs=2,
                                                        name=f"mgst{d0}_{ct}_{f}")
                                        nc.scalar.copy(out=stg[:, 0:cnt],
                                                       in_=ps[:, 0:cnt])
                                        nc.gpsimd.tensor_tensor(
                                            out=view3[:, 0:cnt, c0],
                                            in0=view3[:, 0:cnt, c0],
                                            in1=stg[:, 0:cnt], op=OP.add)
                            return wq

                        wq0 = merge_half(0)
                        xt = []
                        for k in range(8):
                            x_t = xtpool.tile([128, T], BF16, tag=f"xt{k}",
                                              name=f"xt{k}")
                            nc.sync.dma_start(out=x_t[:, :],
                                              in_=xT[128 * k:128 * (k + 1), :])
                            xt.append(x_t)

                        def p2_half(H, wq):
                            for il in range(8):
                                g = 8 * H + il
                                xa_g = xapool.tile([128, T], BF16, tag=f"xa{g}",
                                                   name=f"xa{g}")
                                # two interleaved chains (one per 512-token
                                # chunk) hide the per-chain ldweights bubbles
                                pss = [psA.tile([128, 512], F32, tag="mm",
                                                name=f"xaps{g}_{tch}")
                                       for tch in range(TQ)]
                                for k in range(8):
                                    for tch in range(TQ):
                                        nc.tensor.matmul(
                                            pss[tch][:, :],
                                            wq[k][:, 128 * il:128 * (il + 1)],
                                            xt[k][:, 512 * tch:512 * (tch + 1)],
                                            start=(k == 0), stop=(k == 7))
                                for tch in range(TQ):
                                    nc.scalar.copy(out=xa_g[:, 512 * tch:512 * (tch + 1)],
                                                   in_=pss[tch][:, :])
                                bnstat = misc.tile([128, TQ, 6], F32, tag="bnstat",
                                                   bufs=2, name=f"bnstat{g}")
                                for j in range(TQ):
                                    nc.vector.bn_stats(out=bnstat[:, j, :],
                                                       in_=xa_g[:, 512 * j:512 * (j + 1)])
                                nc.vector.bn_aggr(out=qk_mv[:, g, :], in_=bnstat[:, :, :])
                                xa[g] = xa_g

                        p2_half(0, wq0)          # q channels d in [0, 1024)

                        # -------- WEFF: this core's 128-col shard of
                        # W_eff^T = Wp^T @ Wmp^T with Wmp = Wp + lpA@lpB
                        with tc.tile_pool(name=f"wefc{rep}", bufs=1) as wefc:
                            lpa_sb = wefc.tile([R, C], BF16)
                            nc.sync.dma_start(out=lpa_sb[:, :], in_=lpaT[:, :])
                            lpbn = wefc.tile([128, 8 * R], BF16)
                            for et in range(8):
                                nc.sync.dma_start(
                                    out=lpbn[:, R * et:R * (et + 1)],
                                    in_=lpbN[128 * et:128 * (et + 1), :])
                            wpn = wefc.tile([128, C], BF16)
                            for et in range(8):
                                nc.sync.dma_start(
                                    out=wpn[:, 128 * et:128 * (et + 1)],
                                    in_=wpN[128 * et:128 * (et + 1), :])
                            z_sb = wefc.tile([R, 128], BF16)
                            ps = psA.tile([128, 512], F32, tag="mm", name="zps")
                            for et in range(8):
                                nc.tensor.matmul(
                                    ps[0:R, 0:128],
                                    lpbn[:, R * et:R * (et + 1)],
                                    wpn[:, 128 * et:128 * (et + 1)],
                                    start=(et == 0), stop=(et == 7))
                            nc.scalar.copy(out=z_sb[:, :], in_=ps[0:R, 0:128])
                            # full wpT rows [128, 1024] per e-tile feed both
                            # fc chains from one DMA
                            w2f = []
                            for et in range(8):
                                w2 = wefc.tile([128, C], BF16, tag=f"wpt{et}",
                                               bufs=1, name=f"wpt{et}")
                                nc.sync.dma_start(
                                    out=w2[:, :],
                                    in_=wpT[128 * et:128 * (et + 1), :])
                                w2f.append(w2)
                            psw = [psA.tile([128, 512], F32, tag="mm",
                                            name=f"weffps{fc}") for fc in range(2)]
                            for et in range(8):
                                for fc in range(2):
                                    nc.tensor.matmul(
                                        psw[fc][:, :],
                                        wpn[:, 128 * et:128 * (et + 1)],
                                        w2f[et][:, 512 * fc:512 * (fc + 1)],
                                        start=(et == 0), stop=False)
                            for fc in range(2):
                                nc.tensor.matmul(
                                    psw[fc][:, :], z_sb[:, :],
                                    lpa_sb[:, 512 * fc:512 * (fc + 1)],
                                    start=False, stop=True)
                                wst = wefc.tile([128, 512], wdt, tag="wst", bufs=2,
                                                name=f"weffst{fc}")
                                if FP8_P7:
                                    # prescale into fp8e4 normal range; the
                                    # P7 drain divides it back out
                                    nc.scalar.activation(
                                        out=wst[:, :], in_=psw[fc][:, :],
                                        func=ACTF.Copy, scale=WSCALE)
                                else:
                                    nc.vector.tensor_copy(out=wst[:, :],
                                                          in_=psw[fc][:, :])
                                nc.sync.dma_start(
                                    out=bass.AP(tensor=weff_in[:].tensor,
                                                offset=512 * fc,
                                                ap=[[C, 128], [1, 512]]),
                                    in_=wst[:, :])
                        if single_core or no_collective:
                            # local fallback: replicate shard into all 8 slots
                            for ct in range(8):
                                nc.sync.dma_start(
                                    out=weff_out[128 * C * ct:128 * C * (ct + 1)],
                                    in_=weff_in[:])
                        else:
                            nc.gpsimd.collective_compute(
                                "AllGather", OP.bypass,
                                replica_groups=[list(range(NCORES))],
                                ins=[weff_in[:]], outs=[weff_out[:]])

                        wq1 = merge_half(1024)
                        p2_half(1, wq1)          # k channels d in [1024, 2048)

                        # qk stats -> (mean, E[x^2]) packed, DMA to stats_in
                        nc.vector.tensor_tensor(out=m16[:, :], in0=qk_mv[:, :, 0],
                                                in1=qk_mv[:, :, 0], op=OP.mult)
                        nc.vector.tensor_tensor(out=qk_mv[:, :, 1], in0=qk_mv[:, :, 1],
                                                in1=m16[:, :], op=OP.add)
                        nc.sync.dma_start(
                            out=stats_in[0:4096].rearrange("(p i s) -> p i s", p=128, s=2),
                            in_=qk_mv[:, :, :])
                        all_reduce(stats_in[:], stats_out[:])

                        # W_eff readback: issue now so it drains during P3,
                        # far from the P5/P6 seam.
                        if FP8_P7:
                            # DoubleRow layout [c-part, 2, f]: channel
                            # c = 256*ct2 + 128*i + p
                            for ct2 in range(4):
                                weff[ct2] = projp.tile([128, 2, C], F8,
                                                       tag=f"wf{ct2}",
                                                       name=f"wf{ct2}")
                                nc.sync.dma_start(
                                    out=weff[ct2][:, :, :],
                                    in_=weff_out[256 * C * ct2:256 * C * (ct2 + 1)]
                                    .rearrange("(i p f) -> p i f", i=2, p=128))
                        else:
                            for ct in range(8):
                                weff[ct] = projp.tile([128, C], BF16, tag=f"wf{ct}",
                                                      name=f"wf{ct}")
                                nc.sync.dma_start(
                                    out=weff[ct][:, :],
                                    in_=weff_out[128 * C * ct:128 * C * (ct + 1)]
                                    .rearrange("(p i) -> p i", p=128))
                        # gam/bet are uploaded p-major (gamP[p*24+i] =
                        # gamma[i*128+p]) so every readback is contiguous
                        # per partition instead of a 4-byte-descriptor bomb
                        gv8 = misc.tile([128, 8], F32)
                        nc.sync.dma_start(
                            out=gv8[:, :],
                            in_=bass.AP(tensor=gam[:].tensor, offset=16,
                                        ap=[[24, 128], [1, 8]]))
                        bv8 = misc.tile([128, 8], F32)
                        nc.sync.dma_start(
                            out=bv8[:, :],
                            in_=bass.AP(tensor=bet[:].tensor, offset=16,
                                        ap=[[24, 128], [1, 8]]))

                        # ---------------- P3: v natural + stats ----------------
                        with tc.tile_pool(name=f"psV{rep}", bufs=1, space="PSUM") as psV:
                            wqv = merge_half(2048)

                            # qk-stats readback + normalize: all Vector-engine
                            # so nothing fences the Scalar P3 drain queue;
                            # runs as soon as the AllReduce lands.
                            gqk = misc.tile([128, 16], F32)
                            nc.sync.dma_start(
                                out=gqk[:, :],
                                in_=bass.AP(tensor=gam[:].tensor, offset=0,
                                            ap=[[24, 128], [1, 16]]))
                            bqk = misc.tile([128, 16], F32)
                            nc.sync.dma_start(
                                out=bqk[:, :],
                                in_=bass.AP(tensor=bet[:].tensor, offset=0,
                                            ap=[[24, 128], [1, 16]]))
                            ar_qk = misc.tile([128, 16, 2], F32)
                            nc.sync.dma_start(
                                out=ar_qk[:, :, :],
                                in_=stats_out[0:4096].rearrange("(p i s) -> p i s", p=128, s=2))
                            # q,k: a = gamma*rstd, b = beta - mean*a
                            nc.vector.tensor_scalar(out=ar_qk[:, :, 0], in0=ar_qk[:, :, 0],
                                                    scalar1=1.0 / NCORES, scalar2=None, op0=OP.mult)
                            nc.vector.tensor_scalar(out=ar_qk[:, :, 1], in0=ar_qk[:, :, 1],
                                                    scalar1=1.0 / NCORES, scalar2=None, op0=OP.mult)
                            nc.vector.tensor_tensor(out=m16[:, :], in0=ar_qk[:, :, 0],
                                                    in1=ar_qk[:, :, 0], op=OP.mult)
                            nc.vector.tensor_tensor(out=m16[:, :], in0=ar_qk[:, :, 1],
                                                    in1=m16[:, :], op=OP.subtract)

                            def finish_norm():
                                # emitted mid-P3 so the Scalar queue reaches the
                                # Sqrt only after the AllReduce has landed
                                nc.scalar.activation(out=m16[:, :], in_=m16[:, :],
                                                     func=ACTF.Sqrt, bias=eps_t[:, 0:1])
                                nc.vector.reciprocal(out=m16[:, :], in_=m16[:, :])
                                nc.vector.tensor_tensor(out=qa[:, :], in0=m16[:, :],
                                                        in1=gqk[:, :], op=OP.mult)
                                nc.vector.tensor_tensor(out=qb[:, :], in0=ar_qk[:, :, 0],
                                                        in1=qa[:, :], op=OP.mult)
                                nc.vector.tensor_tensor(out=qb[:, :], in0=bqk[:, :],
                                                        in1=qb[:, :], op=OP.subtract)
                                for g in range(16):
                                    nc.vector.tensor_scalar(
                                        out=xa[g][:, :], in0=xa[g][:, :],
                                        scalar1=qa[:, g:g + 1], scalar2=qb[:, g:g + 1],
                                        op0=OP.mult, op1=OP.add)

                            # v stats: accumulate sum(v) on GpSimd and
                            # sum(v^2) on Vector across token tiles (f32r),
                            # reduce over partitions with 4 ones-matmuls at
                            # the end -> v AllReduce fires at P3 end.
                            acc_v = [None, None]
                            acc_q = [None, None]
                            for tt in range(NT):
                                vnat[tt] = vpool.tile([128, C], BF16,
                                                      tag=f"v{tt}", name=f"v{tt}")
                                pss = [psA.tile([128, 512], F32, tag="mm",
                                                name=f"vps{hc}_{tt}")
                                       for hc in range(2)]
                                for k in range(8):
                                    for hc in range(2):
                                        nc.tensor.matmul(
                                            pss[hc][:, :],
                                            xt[k][:, 128 * tt:128 * (tt + 1)],
                                            wqv[k][:, 512 * hc:512 * (hc + 1)],
                                            start=(k == 0), stop=(k == 7))
                                for hc in range(2):
                                    vsl = vnat[tt][:, 512 * hc:512 * (hc + 1)]
                                    nc.scalar.copy(out=vsl, in_=pss[hc][:, :])
                                    sq = misc.tile([128, 512], BF16, tag="sq", bufs=3,
                                                   name=f"sq{hc}_{tt}")
                                    nc.scalar.activation(
                                        out=sq[:, :], in_=pss[hc][:, :], func=ACTF.Square)
                                    if tt == 0:
                                        acc_v[hc] = misc.tile([128, 512], F32R,
                                                              tag=f"accv{hc}", bufs=1,
                                                              name=f"accv{hc}")
                                        acc_q[hc] = misc.tile([128, 512], F32R,
                                                              tag=f"accq{hc}", bufs=1,
                                                              name=f"accq{hc}")
                                        nc.vector.tensor_copy(out=acc_v[hc][:, :], in_=vsl)
                                        nc.vector.tensor_copy(out=acc_q[hc][:, :], in_=sq[:, :])
                                    else:
                                        nc.vector.tensor_tensor(
                                            out=acc_v[hc][:, :], in0=acc_v[hc][:, :],
                                            in1=vsl, op=OP.add)
                                        nc.vector.tensor_tensor(
                                            out=acc_q[hc][:, :], in0=acc_q[hc][:, :],
                                            in1=sq[:, :], op=OP.add)
                            # after ALL P3 drains so the AllReduce-gated
                            # Sqrt can never stall the Scalar drain queue
                            finish_norm()

                            for hc in range(2):
                                ps_v = psV.tile([1, 512], F32, tag=f"fv{hc}",
                                                name=f"psfv{hc}")
                                nc.tensor.matmul(ps_v[0:1, :], ones_r[:, :],
                                                 acc_v[hc][:, :], start=True, stop=True)
                                vst1 = misc.tile([1, 512], F32, tag="vst", bufs=4,
                                                 name=f"vst1_{hc}")
                                nc.scalar.copy(out=vst1[0:1, :], in_=ps_v[0:1, :])
                                nc.sync.dma_start(
                                    out=vstats_in[512 * hc:512 * (hc + 1)],
                                    in_=vst1[0:1, :])
                                ps_q = psV.tile([1, 512], F32, tag=f"fq{hc}",
                                                name=f"psfq{hc}")
                                nc.tensor.matmul(ps_q[0:1, :], ones_r[:, :],
                                                 acc_q[hc][:, :], start=True, stop=True)
                                vst2 = misc.tile([1, 512], F32, tag="vst", bufs=4,
                                                 name=f"vst2_{hc}")
                                nc.scalar.copy(out=vst2[0:1, :], in_=ps_q[0:1, :])
                                nc.sync.dma_start(
                                    out=vstats_in[C + 512 * hc:C + 512 * (hc + 1)],
                                    in_=vst2[0:1, :])
                            all_reduce(vstats_in[:], vstats_out[:])
                            # v-stats readback emitted here so its many tiny
                            # descriptors drain mid-P5 (right after the
                            # AllReduce lands), not at the P5/P6 seam
                            vs_m = misc.tile([128, 8], F32)
                            nc.sync.dma_start(
                                out=vs_m[:, :],
                                in_=vstats_out[0:C].rearrange("(i p) -> p i", p=128))
                            vs_e = misc.tile([128, 8], F32)
                            nc.sync.dma_start(
                                out=vs_e[:, :],
                                in_=vstats_out[C:2 * C].rearrange("(i p) -> p i", p=128))

            # lorap/xtpool/wbp closed; their SBUF is free for P5 tiles.
            # two independent 1/r broadcast tiles so the early P6 drains
            # never wait on the second broadcast DMA
            r_bc = [projp.tile([128, T // 2], F32, tag=f"rbc{pr}", name=f"rbc{pr}")
                    for pr in range(2)]
            with tc.tile_pool(name=f"bc{rep}", bufs=1) as bcp:
                    rstage = bcp.tile([128, T], F32)   # row 0: r staging
                    # ------------ P5: scores^T, exp, causal, row sums ----
                    # 256-token chunks: finer causal skipping than 512.
                    ae = {}
                    scale = 1.0 / float(np.sqrt(C))
                    with tc.tile_pool(name=f"psR{rep}", bufs=1, space="PSUM") as psR:
                        items = []
                        for tc4 in range(TC4):
                            acts = [st for st in range(NT) if 128 * st < 256 * (tc4 + 1)]
                            for ii, st in enumerate(acts):
                                items.append((tc4, st, ii, len(acts)))
                        # two chunks share one [1,512] bank (half each);
                        # safe: chains are sequential in PE order and the
                        # drain happens after both halves' chains finish
                        ps_rs = {pr: psR.tile([1, 512], F32, tag=f"r{pr}",
                                              name=f"psr{pr}") for pr in range(TC4 // 2)}

                        def r_chain(pr):
                            # both halves of ps_rs[pr] are complete: 1/r via
                            # a [128,4] reshape bounce (DVE reciprocal is
                            # free-dim paced, so [1,512] would cost 3.3us)
                            nc.scalar.copy(
                                out=rstage[0:1, 512 * pr:512 * (pr + 1)],
                                in_=ps_rs[pr][0:1, :])
                            nc.sync.dma_start(
                                out=rb_dram[512 * pr:512 * (pr + 1)],
                                in_=rstage[0:1, 512 * pr:512 * (pr + 1)])
                            # p-major contiguous mapping (order is irrelevant
                            # for a pointwise reciprocal): 64B descriptors
                            # instead of a 512x4B scatter
                            rp = bcp.tile([32, 16], F32, tag="rp",
                                          bufs=2, name=f"rp{pr}")
                            nc.sync.dma_start(
                                out=rp[:, :],
                                in_=rb_dram[512 * pr:512 * (pr + 1)].rearrange(
                                    "(p i) -> p i", p=32))
                            nc.vector.reciprocal(out=rp[:, :], in_=rp[:, :])
                            nc.sync.dma_start(
                                out=rb2_dram[512 * pr:512 * (pr + 1)].rearrange(
                                    "(p i) -> p i", p=32),
                                in_=rp[:, :])
                            nc.sync.dma_start(
                                out=r_bc[pr][:, :],
                                in_=bcast_dram(rb2_dram, 512 * pr, 512))

                        for p0 in range(0, len(items), 2):
                            pair = items[p0:p0 + 2]
                            pss = []
                            for (tc4, st, ii, na) in pair:
                                pss.append(psA.tile([128, 512], F32, tag="mm",
                                                    name=f"scps{tc4}_{st}"))
                            for j in range(8):
                                for pi, (tc4, st, ii, na) in enumerate(pair):
                                    nc.tensor.matmul(
                                        pss[pi][:, 0:256],
                                        xa[8 + j][:, 128 * st:128 * (st + 1)],
                                        xa[j][:, 256 * tc4:256 * (tc4 + 1)],
                                        start=(j == 0), stop=(j == 7))
                            for pi, (tc4, st, ii, na) in enumerate(pair):
                                a_t = attp.tile([128, 256], BF16, tag=f"ae{tc4}_{st}",
                                                name=f"ae{tc4}_{st}")
                                nc.scalar.activation(out=a_t[:, :], in_=pss[pi][:, 0:256],
                                                     func=ACTF.Exp, scale=scale)
                                base = 256 * tc4 - 128 * st
                                if base < 127:
                                    nc.gpsimd.affine_select(
                                        out=a_t[:, :], in_=a_t[:, :],
                                        pattern=[[1, 256]], base=base,
                                        channel_multiplier=-1,
                                        compare_op=OP.is_ge, fill=0.0)
                                pr, half = tc4 // 2, tc4 % 2
                                nc.tensor.matmul(
                                    ps_rs[pr][0:1, 256 * half:256 * (half + 1)],
                                    ones_b[:, :], a_t[:, :],
                                    start=(ii == 0), stop=(ii == na - 1))
                                ae[(tc4, st)] = a_t
                                if ii == na - 1 and half == 1:
                                    r_chain(pr)

            # -------- v scale/bias math (stats already read back) ----
            m8 = misc.tile([128, 8], F32)
            va = misc.tile([128, 8], F32)
            vb = misc.tile([128, 8], F32)
            inv_n = 1.0 / ((1 if single_core else NCORES) * T)

            def vavb_math():
                nc.vector.tensor_scalar(out=vs_m[:, :], in0=vs_m[:, :],
                                        scalar1=inv_n, scalar2=None, op0=OP.mult)
                nc.vector.tensor_scalar(out=vs_e[:, :], in0=vs_e[:, :],
                                        scalar1=inv_n, scalar2=None, op0=OP.mult)
                nc.vector.tensor_tensor(out=m8[:, :], in0=vs_m[:, :], in1=vs_m[:, :], op=OP.mult)
                nc.vector.tensor_tensor(out=m8[:, :], in0=vs_e[:, :], in1=m8[:, :], op=OP.subtract)
                nc.scalar.activation(out=m8[:, :], in_=m8[:, :], func=ACTF.Sqrt,
                                     bias=eps_t[:, 0:1])
                nc.vector.reciprocal(out=m8[:, :], in_=m8[:, :])
                nc.vector.tensor_tensor(out=va[:, :], in0=m8[:, :], in1=gv8[:, :], op=OP.mult)
                nc.vector.tensor_tensor(out=vb[:, :], in0=vs_m[:, :], in1=va[:, :], op=OP.mult)
                nc.vector.tensor_tensor(out=vb[:, :], in0=bv8[:, :], in1=vb[:, :], op=OP.subtract)

            # ---------------- P6: AV + fused BN-v on the drain ---------
            # y = (AV * 1/r) * va[c] + vb[c]. Chains pair two tc4 chunks at
            # the SAME (st, c0) so consecutive matmuls share the stationary
            # operand -- the N=256 ldweights then amortizes over two
            # matmuls instead of being exposed every matmul.
            vavb_math()
            y = [None] * 8
            for ga, gb in ((0, 1), (2, 3)):
                acts_a = list(range(2 * ga + 2))
                acts_b = list(range(2 * gb + 2))
                for c0 in range(8):
                    pss = {tg: psA.tile([128, 512], F32, tag="mm",
                                        name=f"avps{tg}_{c0}") for tg in (ga, gb)}
                    for st in acts_b:
                        for tg in (ga, gb):
                            if st > 2 * tg + 1:
                                continue
                            nc.tensor.matmul(
                                pss[tg][:, 0:256],
                                vnat[st][:, 128 * c0:128 * (c0 + 1)],
                                ae[(tg, st)][:, :],
                                start=(st == 0), stop=(st == 2 * tg + 1))
                    for tg in (ga, gb):
                        if FP8_P7:
                            ct2, iy = c0 // 2, c0 % 2
                            if y[ct2] is None:
                                y[ct2] = projp.tile([128, 2, T], F8,
                                                    tag=f"y{ct2}", name=f"y{ct2}")
                            ysl = y[ct2][:, iy, 256 * tg:256 * (tg + 1)]
                        else:
                            if y[c0] is None:
                                y[c0] = projp.tile([128, T], BF16, tag=f"y{c0}",
                                                   name=f"y{c0}")
                            ysl = y[c0][:, 256 * tg:256 * (tg + 1)]
                        ytmp = misc.tile([128, 256], BF16, tag="ytmp", bufs=3,
                                         name=f"ytmp{tg}_{c0}")
                        nc.vector.tensor_tensor(
                            out=ytmp[:, :], in0=pss[tg][:, 0:256],
                            in1=r_bc[tg // 2][:, 256 * (tg % 2):256 * (tg % 2 + 1)],
                            op=OP.mult)
                        nc.vector.tensor_scalar(
                            out=ysl, in0=ytmp[:, :],
                            scalar1=va[:, c0:c0 + 1], scalar2=vb[:, c0:c0 + 1],
                            op0=OP.mult, op1=OP.add)

            # ------------ P7: single projection via W_eff ----------------
            with tc.tile_pool(name=f"psP{rep}", bufs=2, space="PSUM") as psP:
              for tch in range(TQ):
                for f0 in range(0, 8, 2):
                    pss = [psP.tile([128, 512], F32, tag=f"pp{pi}", bufs=2,
                                    name=f"p2ps{tch}_{f0 + pi}") for pi in range(2)]
                    if FP8_P7:
                        # fp8e4 DoubleRow: each matmul contracts two
                        # 128-channel slices (FD=512 -> ~1.5x regime)
                        for ct2 in range(4):
                            for pi in range(2):
                                nc.tensor.matmul(
                                    pss[pi][:, :],
                                    weff[ct2][:, :, 128 * (f0 + pi):128 * (f0 + pi + 1)],
                                    y[ct2][:, :, 512 * tch:512 * (tch + 1)],
                                    start=(ct2 == 0), stop=(ct2 == 3),
                                    perf_mode=mybir.MatmulPerfMode.DoubleRow)
                    else:
                        for ct in range(8):
                            for pi in range(2):
                                nc.tensor.matmul(
                                    pss[pi][:, :],
                                    weff[ct][:, 128 * (f0 + pi):128 * (f0 + pi + 1)],
                                    y[ct][:, 512 * tch:512 * (tch + 1)],
                                    start=(ct == 0), stop=(ct == 7))
                    for pi in range(2):
                        ft = f0 + pi
                        o_t = outst.tile([128, 512], BF16, tag="o", name=f"o{tch}_{ft}")
                        if FP8_P7:
                            nc.scalar.activation(out=o_t[:, :], in_=pss[pi][:, :],
                                                 func=ACTF.Copy, scale=1.0 / WSCALE)
                        else:
                            nc.scalar.copy(out=o_t[:, :], in_=pss[pi][:, :])
                        nc.sync.dma_start(
                            out=out[128 * ft:128 * (ft + 1), 512 * tch:512 * (tch + 1)],
                            in_=o_t[:, :])


_NC_CACHE = {}


def _get_nc(T):
    if T not in _NC_CACHE:
        _NC_CACHE[T] = build(T)
    return _NC_CACHE[T]


LAST_RESULTS = None
LAST_IN_MAPS = None


def make_in_maps(inputs):
    f = np.float32
    bf = BF16NP
    x = np.asarray(inputs["x"], f)
    B = x.shape[0]
    wT = np.ascontiguousarray(np.asarray(inputs["W_attn"], f).T.astype(bf))  # [C, 3C]
    wp = np.asarray(inputs["W_proj"], f)
    wpT = np.ascontiguousarray(wp.T.astype(bf))                              # [C, C]
    laT = np.ascontiguousarray(np.asarray(inputs["lora_attn_A"], f).T.astype(bf))   # [R, C]
    lbB = np.ascontiguousarray(np.asarray(inputs["lora_attn_B"], f).astype(bf))     # [R, 3C]
    lpaT = np.ascontiguousarray(np.asarray(inputs["lora_proj_A"], f).T.astype(bf))  # [R, C]
    lpbN = np.ascontiguousarray(np.asarray(inputs["lora_proj_B"], f).T.astype(bf))  # [C, R]
    # p-major permutation: gamP[p*24 + i] = gamma[i*128 + p] so on-device
    # readbacks are contiguous per partition
    gam = np.ascontiguousarray(
        np.asarray(inputs["bn_gamma"], f).reshape(24, 128).T.ravel())
    bet = np.ascontiguousarray(
        np.asarray(inputs["bn_beta"], f).reshape(24, 128).T.ravel())
    in_maps = []
    for b in range(B):
        in_maps.append({
            "xT": np.ascontiguousarray(x[b].T.astype(bf)),
            "wT": wT, "wpT": wpT,
            "wpN": np.ascontiguousarray(wp[:, 128 * b:128 * (b + 1)].astype(bf)),
            "laT": laT, "lbB": lbB,
            "lpaT": lpaT, "lpbN": lpbN, "gam": gam, "bet": bet,
        })
    return in_maps


def kernel(x, W_attn, W_proj, lora_attn_A, lora_attn_B, lora_proj_A, lora_proj_B,
           bn_gamma, bn_beta):
    global LAST_RESULTS, LAST_IN_MAPS
    f = np.float32
    x = np.asarray(x, f)
    B, T, C_ = x.shape
    assert C_ == C and B == NCORES

    in_maps = make_in_maps({
        "x": x, "W_attn": W_attn, "W_proj": W_proj,
        "lora_attn_A": lora_attn_A, "lora_attn_B": lora_attn_B,
        "lora_proj_A": lora_proj_A, "lora_proj_B": lora_proj_B,
        "bn_gamma": bn_gamma, "bn_beta": bn_beta})

    LAST_IN_MAPS = in_maps
    nc = _get_nc(T)
    res = run_bass_kernel_spmd(nc, in_maps, core_ids=list(range(NCORES)))
    LAST_RESULTS = res
    return np.stack([np.asarray(res.results[b]["out"]).T for b in range(B)]).astype(f)


# revision 36
# speedup vs baseline: 1.0289x; 1.0289x over previous
"""Trainium2 Bass kernel for nn_Attention_4_lora (B=8, T=1024, C=1024, R=64).

Strategy: data-parallel over the batch dim (1 batch per NeuronCore, 8 cores).
All activations live in transposed [channel, token] layout so that every
matmul contraction runs over the SBUF partition axis. BatchNorm statistics
are reduced across cores with two small AllReduces. All matmul operands are
bf16 (full PE rate, FWL fast-weight-load, half the SBUF/DMA traffic of
f32r); accumulation stays fp32 in PSUM.

Per-core pipeline:
  P1  merge Wm_attn^T = W_attn^T + reshape(A@B)^T on device, in d-HALVES.
      The K=64 LoRA delta matmuls run PAIRED via 2x row tiling (operands
      duplicated on partitions 0-63 / 64-127), so two deltas share the PE
      array. The strided adds alternate between Vector and GpSimd.
  WEFF (between the P2 halves) this core's 128-row shard of
      W_eff^T = Wp^T @ Wmp^T  (Wmp = Wp + lpA@lpB), via
      Z = lpB @ Wp[:, shard]  then  shard = Wp[:, shard]^T Wp^T + Z^T lpA^T
      -- all chained N=512 matmuls; bf16 AllGather of the 8 shards ->
      every core holds the full [C, C] W_eff^T. The readback DMAs are
      issued right after the qk AllReduce so the 2MB (1024-descriptor)
      transfer drains during P3, far away from the P5/P6 seam.
  P2  xa^T[d, t] = Wm^T-slab.T @ x^T for q (half 0) and k (half 1),
      bn_stats per tile
  P3  v[t, c] (natural layout, needed as AV stationary); v stats are
      accumulated across token tiles on GpSimd (sum v) and Vector (sum v^2)
      in f32r, then reduced over the partition axis with 4 ones-matmuls;
      the v AllReduce fires at P3 end.
  P5  scores^T[s, t] in 256-token chunks (finer causal skipping),
      exp((q.k)/32) on ScalarE, causal mask via affine_select, row-sums
      via ones-matmul into half-banks; 1/r via a [128,4] reshape bounce
      (DVE reciprocal is free-dim paced).
  P6  y^T[c, t] = v-slab.T @ att_exp^T; drain fuses 1/r (Vector) and the
      BN-v scale/bias y = y*va[c] + vb[c] (GpSimd), so P7 needs no bias.
  P7  single projection: out^T = W_eff^T-slab.T @ y^T -> [C, T] in bf16.

kernel() takes the full unsharded inputs, shards/uploads (weights cast to
bf16 on host), runs SPMD on cores 0-7, gathers, and transposes back to
[B, T, C].
"""

import ml_dtypes
import numpy as np

import concourse.bass as bass
import concourse.mybir as mybir
import concourse.tile as tile
from concourse import bacc
from concourse.bass_utils import run_bass_kernel_spmd

NCORES = 8
C = 1024
R = 64
D3 = 3 * C
EPS = 1e-5
F32 = mybir.dt.float32
F32R = mybir.dt.float32r
BF16 = mybir.dt.bfloat16
F8 = mybir.dt.float8e4
FP8_P7 = False     # fp8e4 DoubleRow P7 measured 3.6e-2 rel err (> 2e-2 gate)
WSCALE = 1024.0    # W_eff prescale into the fp8e4 normal range
AX = mybir.AxisListType
OP = mybir.AluOpType
ACTF = mybir.ActivationFunctionType
BF16NP = ml_dtypes.bfloat16


def _erange(f, d0, d1):
    """e-range such that d = 3e + f lies in [d0, d1)."""
    el = -((-(d0 - f)) // 3)
    eh = -((-(d1 - f)) // 3)
    return el, eh


def build(T=1024, single_core=False, no_collective=False, reps=1):
    NT = T // 128          # 128-token tiles
    TQ = T // 512          # 512-token chunks
    assert T % 512 == 0

    nc = bacc.Bacc(None, target_bir_lowering=False,
                   num_devices=(1 if single_core else NCORES))

    prm = {}
    prm["xT"] = nc.declare_dram_parameter("xT", [C, T], BF16, isOutput=False)
    prm["wT"] = nc.declare_dram_parameter("wT", [C, D3], BF16, isOutput=False)
    prm["wpT"] = nc.declare_dram_parameter("wpT", [C, C], BF16, isOutput=False)
    prm["wpN"] = nc.declare_dram_parameter("wpN", [C, 128], BF16, isOutput=False)
    prm["laT"] = nc.declare_dram_parameter("laT", [R, C], BF16, isOutput=False)
    prm["lbB"] = nc.declare_dram_parameter("lbB", [R, D3], BF16, isOutput=False)
    prm["lpaT"] = nc.declare_dram_parameter("lpaT", [R, C], BF16, isOutput=False)
    prm["lpbN"] = nc.declare_dram_parameter("lpbN", [C, R], BF16, isOutput=False)
    prm["gam"] = nc.declare_dram_parameter("gam", [D3], F32, isOutput=False)
    prm["bet"] = nc.declare_dram_parameter("bet", [D3], F32, isOutput=False)
    prm["out"] = nc.declare_dram_parameter("out", [C, T], BF16, isOutput=True)

    with tile.TileContext(nc) as tc:
        for rep in range(reps):
            _emit(nc, tc, prm, T, rep, single_core, no_collective)

    nc.compile()
    return nc


def _emit(nc, tc, prm, T, rep, single_core, no_collective):
    NT = T // 128
    TQ = T // 512
    TC4 = T // 256
    xT, wT, wpT, wpN = prm["xT"], prm["wT"], prm["wpT"], prm["wpN"]
    laT, lbB = prm["laT"], prm["lbB"]
    lpaT, lpbN, gam, bet, out = prm["lpaT"], prm["lpbN"], prm["gam"], prm["bet"], prm["out"]

    stats_in = nc.dram_tensor(f"stats_in_{rep}", [4096], F32)
    stats_out = nc.dram_tensor(f"stats_out_{rep}", [4096], F32)
    vstats_in = nc.dram_tensor(f"vstats_in_{rep}", [2 * C], F32)
    vstats_out = nc.dram_tensor(f"vstats_out_{rep}", [2 * C], F32)
    rb_dram = nc.dram_tensor(f"rb_{rep}", [T], F32)
    rb2_dram = nc.dram_tensor(f"rb2_{rep}", [T], F32)
    wdt = F8 if FP8_P7 else BF16
    weff_in = nc.dram_tensor(f"weff_in_{rep}", [128 * C], wdt)
    weff_out = nc.dram_tensor(f"weff_out_{rep}", [C * C], wdt,
                              addr_space="Shared")

    def bcast_dram(param, offset, n):
        return bass.AP(tensor=param[:].tensor, offset=offset, ap=[[0, 128], [1, n]])

    def all_reduce(ins, outs):
        if single_core or no_collective:
            nc.sync.dma_start(out=outs, in_=ins)
        else:
            nc.gpsimd.collective_compute(
                "AllReduce", OP.add,
                replica_groups=[list(range(NCORES))],
                ins=[ins], outs=[outs])

    with (
        tc.tile_pool(name=f"misc{rep}", bufs=1) as misc,
        tc.tile_pool(name=f"outst{rep}", bufs=2) as outst,
        tc.tile_pool(name=f"vpool{rep}", bufs=1) as vpool,
        tc.tile_pool(name=f"attp{rep}", bufs=1) as attp,
        tc.tile_pool(name=f"psA{rep}", bufs=4, space="PSUM") as psA,
    ):
        # ---------------- constants / small loads ----------------
        ones_f = misc.tile([128, 1], F32)
        nc.vector.memset(ones_f[:, :], 1.0)
        ones_b = misc.tile([128, 1], BF16)
        nc.vector.tensor_copy(out=ones_b[:, :], in_=ones_f[:, :])
        ones_r = misc.tile([128, 1], F32R)
        nc.vector.tensor_copy(out=ones_r[:, :], in_=ones_f[:, :])
        eps_t = misc.tile([128, 1], F32)
        nc.vector.memset(eps_t[:, :], EPS)

        qk_mv = misc.tile([128, 16, 2], F32)
        m16 = misc.tile([128, 16], F32)
        qa = misc.tile([128, 16], F32)
        qb = misc.tile([128, 16], F32)

        xa = [None] * 16
        vnat = [None] * NT
        weff = [None] * 8

        with tc.tile_pool(name=f"xapool{rep}", bufs=1) as xapool:
          with tc.tile_pool(name=f"projp{rep}", bufs=1) as projp:
            with tc.tile_pool(name=f"lorap{rep}", bufs=1) as lorap:
                # la/lb duplicated on partitions 0-63 and 64-127 so the K=64
                # delta matmuls can run pairwise via 2x row tiling; the
                # second copy is an SBUF->SBUF DMA (no HBM bandwidth)
                la2 = lorap.tile([128, C], BF16)
                nc.sync.dma_start(out=la2[0:R, :], in_=laT[:, :])
                nc.sync.dma_start(out=la2[R:2 * R, :], in_=la2[0:R, :])
                lb2 = lorap.tile([128, D3], BF16)
                for _c in range(3):
                    nc.sync.dma_start(out=lb2[0:R, 1024 * _c:1024 * (_c + 1)],
                                      in_=lbB[:, 1024 * _c:1024 * (_c + 1)])
                    nc.sync.dma_start(out=lb2[R:2 * R, 1024 * _c:1024 * (_c + 1)],
                                      in_=lb2[0:R, 1024 * _c:1024 * (_c + 1)])

                with tc.tile_pool(name=f"xtpool{rep}", bufs=1) as xtpool:
                    with tc.tile_pool(name=f"wb{rep}", bufs=1) as wbp:
                        # -------- P1 merge (d-halves, paired deltas) ------
                        def merge_half(d0, interleave=None):
                            """Merged Wm^T[:, d0:d0+1024] as 8 c-tiles [128, 1032]."""
                            wq = []
                            for ct in range(8):
                                w_t = wbp.tile([128, 1032], BF16, tag=f"wb{ct}",
                                               bufs=(2 if ct < 4 else 1),
                                               name=f"wh{d0}_{ct}")
                                nc.sync.dma_start(
                                    out=w_t[:, 0:1024],
                                    in_=wT[128 * ct:128 * (ct + 1), d0:d0 + 1024])
                                if interleave is not None:
                                    interleave(ct)
                                wq.append(w_t)
                            jobs = [(ct, f) for ct in range(8) for f in range(3)]
                            for j0 in range(0, len(jobs), 2):
                                pss = []
                                for pi, (ct, f) in enumerate(jobs[j0:j0 + 2]):
                                    r0 = R * pi      # partition offset 0 / 64
                                    el, eh = _erange(f, d0, d0 + 1024)
                                    cnt = eh - el
                                    ps = psA.tile([128, 512], F32, tag="mm",
                                                  name=f"dps{d0}_{ct}_{f}")
                                    nc.tensor.matmul(
                                        ps[:, 0:cnt],
                                        lb2[r0:r0 + R,
                                            1024 * f + 128 * ct:1024 * f + 128 * (ct + 1)],
                                        la2[r0:r0 + R, el:el + cnt],
                                        start=True, stop=True)
                                    pss.append(ps)
                                for pi, (ct, f) in enumerate(jobs[j0:j0 + 2]):
                                    w_t = wq[ct]
                                    view3 = w_t[:, :].rearrange(
                                        "p (u three) -> p u three", three=3)
                                    el, eh = _erange(f, d0, d0 + 1024)
                                    cnt = eh - el
                                    c0 = 3 * el + f - d0
                                    ps = pss[pi]
                                    if (ct + f) % 2 == 0:
                                        nc.vector.tensor_tensor(
                                            out=view3[:, 0:cnt, c0],
                                            in0=view3[:, 0:cnt, c0],
                                            in1=ps[:, 0:cnt], op=OP.add)
                                    else:
                                        # GpSimd cannot read PSUM: stage via
                                        # Scalar, add on GpSimd
                                        stg = misc.tile([128, 344], BF16,
                                                        tag="mgst", bufs=2,
                                                        name=f"mgst{d0}_{ct}_{f}")
                                        nc.scalar.copy(out=stg[:, 0:cnt],
                                                       in_=ps[:, 0:cnt])
                                        nc.gpsimd.tensor_tensor(
                                            out=view3[:, 0:cnt, c0],
                                            in0=view3[:, 0:cnt, c0],
                                            in1=stg[:, 0:cnt], op=OP.add)
                            return wq

                        wq0 = merge_half(0)
                        xt = []
                        for k in range(8):
                            x_t = xtpool.tile([128, T], BF16, tag=f"xt{k}",
                                              name=f"xt{k}")
                            nc.sync.dma_start(out=x_t[:, :],
                                              in_=xT[128 * k:128 * (k + 1), :])
                            xt.append(x_t)

                        def p2_half(H, wq):
                            for il in range(8):
                                g = 8 * H + il
                                xa_g = xapool.tile([128, T], BF16, tag=f"xa{g}",
                                                   name=f"xa{g}")
                                # two interleaved chains (one per 512-token
                                # chunk) hide the per-chain ldweights bubbles
                                pss = [psA.tile([128, 512], F32, tag="mm",
                                                name=f"xaps{g}_{tch}")
                                       for tch in range(TQ)]
                                for k in range(8):
                                    for tch in range(TQ):
                                        nc.tensor.matmul(
                                            pss[tch][:, :],
                                            wq[k][:, 128 * il:128 * (il + 1)],
                                            xt[k][:, 512 * tch:512 * (tch + 1)],
                                            start=(k == 0), stop=(k == 7))
                                for tch in range(TQ):
                                    nc.scalar.copy(out=xa_g[:, 512 * tch:512 * (tch + 1)],
                                                   in_=pss[tch][:, :])
                                bnstat = misc.tile([128, TQ, 6], F32, tag="bnstat",
                                                   bufs=2, name=f"bnstat{g}")
                                for j in range(TQ):
                                    nc.vector.bn_stats(out=bnstat[:, j, :],
                                                       in_=xa_g[:, 512 * j:512 * (j + 1)])
                                nc.vector.bn_aggr(out=qk_mv[:, g, :], in_=bnstat[:, :, :])
                                xa[g] = xa_g

                        p2_half(0, wq0)          # q channels d in [0, 1024)

                        # -------- WEFF: this core's 128-col shard of
                        # W_eff^T = Wp^T @ Wmp^T with Wmp = Wp + lpA@lpB
                        with tc.tile_pool(name=f"wefc{rep}", bufs=1) as wefc:
                            lpa_sb = wefc.tile([R, C], BF16)
                            nc.sync.dma_start(out=lpa_sb[:, :], in_=lpaT[:, :])
                            lpbn = wefc.tile([128, 8 * R], BF16)
                            for et in range(8):
                                nc.sync.dma_start(
                                    out=lpbn[:, R * et:R * (et + 1)],
                                    in_=lpbN[128 * et:128 * (et + 1), :])
                            wpn = wefc.tile([128, C], BF16)
                            for et in range(8):
                                nc.sync.dma_start(
                                    out=wpn[:, 128 * et:128 * (et + 1)],
                                    in_=wpN[128 * et:128 * (et + 1), :])
                            z_sb = wefc.tile([R, 128], BF16)
                            ps = psA.tile([128, 512], F32, tag="mm", name="zps")
                            for et in range(8):
                                nc.tensor.matmul(
                                    ps[0:R, 0:128],
                                    lpbn[:, R * et:R * (et + 1)],
                                    wpn[:, 128 * et:128 * (et + 1)],
                                    start=(et == 0), stop=(et == 7))
                            nc.scalar.copy(out=z_sb[:, :], in_=ps[0:R, 0:128])
                            # full wpT rows [128, 1024] per e-tile feed both
                            # fc chains from one DMA
                            w2f = []
                            for et in range(8):
                                w2 = wefc.tile([128, C], BF16, tag=f"wpt{et}",
                                               bufs=1, name=f"wpt{et}")
                                nc.sync.dma_start(
                                    out=w2[:, :],
                                    in_=wpT[128 * et:128 * (et + 1), :])
                                w2f.append(w2)
                            psw = [psA.tile([128, 512], F32, tag="mm",
                                            name=f"weffps{fc}") for fc in range(2)]
                            for et in range(8):
                                for fc in range(2):
                                    nc.tensor.matmul(
                                        psw[fc][:, :],
                                        wpn[:, 128 * et:128 * (et + 1)],
                                        w2f[et][:, 512 * fc:512 * (fc + 1)],
                                        start=(et == 0), stop=False)
                            for fc in range(2):
                                nc.tensor.matmul(
                                    psw[fc][:, :], z_sb[:, :],
                                    lpa_sb[:, 512 * fc:512 * (fc + 1)],
                                    start=False, stop=True)
                                wst = wefc.tile([128, 512], wdt, tag="wst", bufs=2,
                                                name=f"weffst{fc}")
                                if FP8_P7:
                                    # prescale into fp8e4 normal range; the
                                    # P7 drain divides it back out
                                    nc.scalar.activation(
                                        out=wst[:, :], in_=psw[fc][:, :],
                                        func=ACTF.Copy, scale=WSCALE)
                                else:
                                    nc.vector.tensor_copy(out=wst[:, :],
                                                          in_=psw[fc][:, :])
                                nc.sync.dma_start(
                                    out=bass.AP(tensor=weff_in[:].tensor,
                                                offset=512 * fc,
                                                ap=[[C, 128], [1, 512]]),
                                    in_=wst[:, :])
                        if single_core or no_collective:
                            # local fallback: replicate shard into all 8 slots
                            for ct in range(8):
                                nc.sync.dma_start(
                                    out=weff_out[128 * C * ct:128 * C * (ct + 1)],
                                    in_=weff_in[:])
                        else:
                            nc.gpsimd.collective_compute(
                                "AllGather", OP.bypass,
                                replica_groups=[list(range(NCORES))],
                                ins=[weff_in[:]], outs=[weff_out[:]])

                        wq1 = merge_half(1024)
                        p2_half(1, wq1)          # k channels d in [1024, 2048)

                        # qk stats -> (mean, E[x^2]) packed, DMA to stats_in
                        nc.vector.tensor_tensor(out=m16[:, :], in0=qk_mv[:, :, 0],
                                                in1=qk_mv[:, :, 0], op=OP.mult)
                        nc.vector.tensor_tensor(out=qk_mv[:, :, 1], in0=qk_mv[:, :, 1],
                                                in1=m16[:, :], op=OP.add)
                        nc.sync.dma_start(
                            out=stats_in[0:4096].rearrange("(p i s) -> p i s", p=128, s=2),
                            in_=qk_mv[:, :, :])
                        all_reduce(stats_in[:], stats_out[:])

                        # W_eff readback: issue now so it drains during P3,
                        # far from the P5/P6 seam.
                        if FP8_P7:
                            # DoubleRow layout [c-part, 2, f]: channel
                            # c = 256*ct2 + 128*i + p
                            for ct2 in range(4):
                                weff[ct2] = projp.tile([128, 2, C], F8,
                                                       tag=f"wf{ct2}",
                                                       name=f"wf{ct2}")
                                nc.sync.dma_start(
                                    out=weff[ct2][:, :, :],
                                    in_=weff_out[256 * C * ct2:256 * C * (ct2 + 1)]
                                    .rearrange("(i p f) -> p i f", i=2, p=128))
                        else:
                            for ct in range(8):
                                weff[ct] = projp.tile([128, C], BF16, tag=f"wf{ct}",
                                                      name=f"wf{ct}")
                                nc.sync.dma_start(
                                    out=weff[ct][:, :],
                                    in_=weff_out[128 * C * ct:128 * C * (ct + 1)]
                                    .rearrange("(p i) -> p i", p=128))
                        # gam/bet are uploaded p-major (gamP[p*24+i] =
                        # gamma[i*128+p]) so every readback is contiguous
                        # per partition instead of a 4-byte-descriptor bomb
                        gv8 = misc.tile([128, 8], F32)
                        nc.sync.dma_start(
                            out=gv8[:, :],
                            in_=bass.AP(tensor=gam[:].tensor, offset=16,
                                        ap=[[24, 128], [1, 8]]))
                        bv8 = misc.tile([128, 8], F32)
                        nc.sync.dma_start(
                            out=bv8[:, :],
                            in_=bass.AP(tensor=bet[:].tensor, offset=16,
                                        ap=[[24, 128], [1, 8]]))

                        # ---------------- P3: v natural + stats ----------------
                        with tc.tile_pool(name=f"psV{rep}", bufs=1, space="PSUM") as psV:
                            wqv = merge_half(2048)

                            # qk-stats readback + normalize: all Vector-engine
                            # so nothing fences the Scalar P3 drain queue;
                            # runs as soon as the AllReduce lands.
                            gqk = misc.tile([128, 16], F32)
                            nc.sync.dma_start(
                                out=gqk[:, :],
                                in_=bass.AP(tensor=gam[:].tensor, offset=0,
                                            ap=[[24, 128], [1, 16]]))
                            bqk = misc.tile([128, 16], F32)
                            nc.sync.dma_start(
                                out=bqk[:, :],
                                in_=bass.AP(tensor=bet[:].tensor, offset=0,
                                            ap=[[24, 128], [1, 16]]))
                            ar_qk = misc.tile([128, 16, 2], F32)
                            nc.sync.dma_start(
                                out=ar_qk[:, :, :],
                                in_=stats_out[0:4096].rearrange("(p i s) -> p i s", p=128, s=2))
                            # q,k: a = gamma*rstd, b = beta - mean*a
                            nc.vector.tensor_scalar(out=ar_qk[:, :, 0], in0=ar_qk[:, :, 0],
                                                    scalar1=1.0 / NCORES, scalar2=None, op0=OP.mult)
                            nc.vector.tensor_scalar(out=ar_qk[:, :, 1], in0=ar_qk[:, :, 1],
                                                    scalar1=1.0 / NCORES, scalar2=None, op0=OP.mult)
                            nc.vector.tensor_tensor(out=m16[:, :], in0=ar_qk[:, :, 0],
                                                    in1=ar_qk[:, :, 0], op=OP.mult)
                            nc.vector.tensor_tensor(out=m16[:, :], in0=ar_qk[:, :, 1],
                                                    in1=m16[:, :], op=OP.subtract)

                            def finish_norm():
                                # emitted mid-P3 so the Scalar queue reaches the
                                # Sqrt only after the AllReduce has landed
                                nc.scalar.activation(out=m16[:, :], in_=m16[:, :],
                                                     func=ACTF.Sqrt, bias=eps_t[:, 0:1])
                                nc.vector.reciprocal(out=m16[:, :], in_=m16[:, :])
                                nc.vector.tensor_tensor(out=qa[:, :], in0=m16[:, :],
                                                        in1=gqk[:, :], op=OP.mult)
                                nc.vector.tensor_tensor(out=qb[:, :], in0=ar_qk[:, :, 0],
                                                        in1=qa[:, :], op=OP.mult)
                                nc.vector.tensor_tensor(out=qb[:, :], in0=bqk[:, :],
                                                        in1=qb[:, :], op=OP.subtract)
                                for g in range(16):
                                    nc.vector.tensor_scalar(
                                        out=xa[g][:, :], in0=xa[g][:, :],
                                        scalar1=qa[:, g:g + 1], scalar2=qb[:, g:g + 1],
                                        op0=OP.mult, op1=OP.add)

                            # v stats: accumulate sum(v) on GpSimd and
                            # sum(v^2) on Vector across token tiles (f32r),
                            # reduce over partitions with 4 ones-matmuls at
                            # the end -> v AllReduce fires at P3 end.
                            acc_v = [None, None]
                            acc_q = [None, None]
                            for tt in range(NT):
                                vnat[tt] = vpool.tile([128, C], BF16,
                                                      tag=f"v{tt}", name=f"v{tt}")
                                pss = [psA.tile([128, 512], F32, tag="mm",
                                                name=f"vps{hc}_{tt}")
                                       for hc in range(2)]
                                for k in range(8):
                                    for hc in range(2):
                                        nc.tensor.matmul(
                                            pss[hc][:, :],
                                            xt[k][:, 128 * tt:128 * (tt + 1)],
                                            wqv[k][:, 512 * hc:512 * (hc + 1)],
                                            start=(k == 0), stop=(k == 7))
                                for hc in range(2):
                                    vsl = vnat[tt][:, 512 * hc:512 * (hc + 1)]
                                    nc.scalar.copy(out=vsl, in_=pss[hc][:, :])
                                    sq = misc.tile([128, 512], BF16, tag="sq", bufs=3,
                                                   name=f"sq{hc}_{tt}")
                                    nc.scalar.activation(
                                        out=sq[:, :], in_=pss[hc][:, :], func=ACTF.Square)
                                    if tt == 0:
                                        acc_v[hc] = misc.tile([128, 512], F32R,
                                                              tag=f"accv{hc}", bufs=1,
                                                              name=f"accv{hc}")
                                        acc_q[hc] = misc.tile([128, 512], F32R,
                                                              tag=f"accq{hc}", bufs=1,
                                                              name=f"accq{hc}")
                                        nc.vector.tensor_copy(out=acc_v[hc][:, :], in_=vsl)
                                        nc.vector.tensor_copy(out=acc_q[hc][:, :], in_=sq[:, :])
                                    else:
                                        nc.vector.tensor_tensor(
                                            out=acc_v[hc][:, :], in0=acc_v[hc][:, :],
                                            in1=vsl, op=OP.add)
                                        nc.vector.tensor_tensor(
                                            out=acc_q[hc][:, :], in0=acc_q[hc][:, :],
                                            in1=sq[:, :], op=OP.add)
                            # after ALL P3 drains so the AllReduce-gated
                            # Sqrt can never stall the Scalar drain queue
                            finish_norm()

                            for hc in range(2):
                                ps_v = psV.tile([1, 512], F32, tag=f"fv{hc}",
                                                name=f"psfv{hc}")
                                nc.tensor.matmul(ps_v[0:1, :], ones_r[:, :],
                                                 acc_v[hc][:, :], start=True, stop=True)
                                vst1 = misc.tile([1, 512], F32, tag="vst", bufs=4,
                                                 name=f"vst1_{hc}")
                                nc.scalar.copy(out=vst1[0:1, :], in_=ps_v[0:1, :])
                                nc.sync.dma_start(
                                    out=vstats_in[512 * hc:512 * (hc + 1)],
                                    in_=vst1[0:1, :])
                                ps_q = psV.tile([1, 512], F32, tag=f"fq{hc}",
                                                name=f"psfq{hc}")
                                nc.tensor.matmul(ps_q[0:1, :], ones_r[:, :],
                                                 acc_q[hc][:, :], start=True, stop=True)
                                vst2 = misc.tile([1, 512], F32, tag="vst", bufs=4,
                                                 name=f"vst2_{hc}")
                                nc.scalar.copy(out=vst2[0:1, :], in_=ps_q[0:1, :])
                                nc.sync.dma_start(
                                    out=vstats_in[C + 512 * hc:C + 512 * (hc + 1)],
                                    in_=vst2[0:1, :])
                            all_reduce(vstats_in[:], vstats_out[:])
                            # v-stats readback emitted here so its many tiny
                            # descriptors drain mid-P5 (right after the
                            # AllReduce lands), not at the P5/P6 seam
                            vs_m = misc.tile([128, 8], F32)
                            nc.sync.dma_start(
                                out=vs_m[:, :],
                                in_=vstats_out[0:C].rearrange("(i p) -> p i", p=128))
                            vs_e = misc.tile([128, 8], F32)
                            nc.sync.dma_start(
                                out=vs_e[:, :],
                                in_=vstats_out[C:2 * C].rearrange("(i p) -> p i", p=128))

            # lorap/xtpool/wbp closed; their SBUF is free for P5 tiles.
            # two independent 1/r broadcast tiles so the early P6 drains
            # never wait on the second broadcast DMA
            r_bc = [projp.tile([128, T // 2], F32, tag=f"rbc{pr}", name=f"rbc{pr}")
                    for pr in range(2)]
            with tc.tile_pool(name=f"bc{rep}", bufs=1) as bcp:
                    rstage = bcp.tile([128, T], F32)   # row 0: r staging
                    # ------------ P5: scores^T, exp, causal, row sums ----
                    # 256-token chunks: finer causal skipping than 512.
                    ae = {}
                    scale = 1.0 / float(np.sqrt(C))
                    with tc.tile_pool(name=f"psR{rep}", bufs=1, space="PSUM") as psR:
                        items = []
                        for tc4 in range(TC4):
                            acts = [st for st in range(NT) if 128 * st < 256 * (tc4 + 1)]
                            for ii, st in enumerate(acts):
                                items.append((tc4, st, ii, len(acts)))
                        # two chunks share one [1,512] bank (half each);
                        # safe: chains are sequential in PE order and the
                        # drain happens after both halves' chains finish
                        ps_rs = {pr: psR.tile([1, 512], F32, tag=f"r{pr}",
                                              name=f"psr{pr}") for pr in range(TC4 // 2)}

                        def r_chain(pr):
                            # both halves of ps_rs[pr] are complete: 1/r via
                            # a [128,4] reshape bounce (DVE reciprocal is
                            # free-dim paced, so [1,512] would cost 3.3us).
                            # All DMAs ride the Scalar engine's HW-DGE queue
                            # (idle at P5 end), decoupled from the Sync
                            # queue's completion-slot chains.
                            nc.scalar.copy(
                                out=rstage[0:1, 512 * pr:512 * (pr + 1)],
                                in_=ps_rs[pr][0:1, :])
                            nc.scalar.dma_start(
                                out=rb_dram[512 * pr:512 * (pr + 1)],
                                in_=rstage[0:1, 512 * pr:512 * (pr + 1)])
                            # p-major contiguous mapping (order is irrelevant
                            # for a pointwise reciprocal): 64B descriptors
                            # instead of a 512x4B scatter
                            rp = bcp.tile([32, 16], F32, tag="rp",
                                          bufs=2, name=f"rp{pr}")
                            nc.scalar.dma_start(
                                out=rp[:, :],
                                in_=rb_dram[512 * pr:512 * (pr + 1)].rearrange(
                                    "(p i) -> p i", p=32))
                            nc.vector.reciprocal(out=rp[:, :], in_=rp[:, :])
                            nc.scalar.dma_start(
                                out=rb2_dram[512 * pr:512 * (pr + 1)].rearrange(
                                    "(p i) -> p i", p=32),
                                in_=rp[:, :])
                            nc.scalar.dma_start(
                                out=r_bc[pr][:, :],
                                in_=bcast_dram(rb2_dram, 512 * pr, 512))

                        for p0 in range(0, len(items), 2):
                            pair = items[p0:p0 + 2]
                            pss = []
                            for (tc4, st, ii, na) in pair:
                                pss.append(psA.tile([128, 512], F32, tag="mm",
                                                    name=f"scps{tc4}_{st}"))
                            for j in range(8):
                                for pi, (tc4, st, ii, na) in enumerate(pair):
                                    nc.tensor.matmul(
                                        pss[pi][:, 0:256],
                                        xa[8 + j][:, 128 * st:128 * (st + 1)],
                                        xa[j][:, 256 * tc4:256 * (tc4 + 1)],
                                        start=(j == 0), stop=(j == 7))
                            for pi, (tc4, st, ii, na) in enumerate(pair):
                                a_t = attp.tile([128, 256], BF16, tag=f"ae{tc4}_{st}",
                                                name=f"ae{tc4}_{st}")
                                nc.scalar.activation(out=a_t[:, :], in_=pss[pi][:, 0:256],
                                                     func=ACTF.Exp, scale=scale)
                                base = 256 * tc4 - 128 * st
                                if base < 127:
                                    nc.gpsimd.affine_select(
                                        out=a_t[:, :], in_=a_t[:, :],
                                        pattern=[[1, 256]], base=base,
                                        channel_multiplier=-1,
                                        compare_op=OP.is_ge, fill=0.0)
                                pr, half = tc4 // 2, tc4 % 2
                                nc.tensor.matmul(
                                    ps_rs[pr][0:1, 256 * half:256 * (half + 1)],
                                    ones_b[:, :], a_t[:, :],
                                    start=(ii == 0), stop=(ii == na - 1))
                                ae[(tc4, st)] = a_t
                                if ii == na - 1 and half == 1:
                                    r_chain(pr)

            # -------- v scale/bias math (stats already read back) ----
            m8 = misc.tile([128, 8], F32)
            va = misc.tile([128, 8], F32)
            vb = misc.tile([128, 8], F32)
            inv_n = 1.0 / ((1 if single_core else NCORES) * T)

            def vavb_math():
                nc.vector.tensor_scalar(out=vs_m[:, :], in0=vs_m[:, :],
                                        scalar1=inv_n, scalar2=None, op0=OP.mult)
                nc.vector.tensor_scalar(out=vs_e[:, :], in0=vs_e[:, :],
                                        scalar1=inv_n, scalar2=None, op0=OP.mult)
                nc.vector.tensor_tensor(out=m8[:, :], in0=vs_m[:, :], in1=vs_m[:, :], op=OP.mult)
                nc.vector.tensor_tensor(out=m8[:, :], in0=vs_e[:, :], in1=m8[:, :], op=OP.subtract)
                nc.scalar.activation(out=m8[:, :], in_=m8[:, :], func=ACTF.Sqrt,
                                     bias=eps_t[:, 0:1])
                nc.vector.reciprocal(out=m8[:, :], in_=m8[:, :])
                nc.vector.tensor_tensor(out=va[:, :], in0=m8[:, :], in1=gv8[:, :], op=OP.mult)
                nc.vector.tensor_tensor(out=vb[:, :], in0=vs_m[:, :], in1=va[:, :], op=OP.mult)
                nc.vector.tensor_tensor(out=vb[:, :], in0=bv8[:, :], in1=vb[:, :], op=OP.subtract)

            # ---------------- P6: AV + fused BN-v on the drain ---------
            # y = (AV * 1/r) * va[c] + vb[c]. Chains pair two tc4 chunks at
            # the SAME (st, c0) so consecutive matmuls share the stationary
            # operand -- the N=256 ldweights then amortizes over two
            # matmuls instead of being exposed every matmul.
            vavb_math()
            y = [None] * 8
            for ga, gb in ((0, 1), (2, 3)):
                acts_a = list(range(2 * ga + 2))
                acts_b = list(range(2 * gb + 2))
                for c0 in range(8):
                    pss = {tg: psA.tile([128, 512], F32, tag="mm",
                                        name=f"avps{tg}_{c0}") for tg in (ga, gb)}
                    for st in acts_b:
                        for tg in (ga, gb):
                            if st > 2 * tg + 1:
                                continue
                            nc.tensor.matmul(
                                pss[tg][:, 0:256],
                                vnat[st][:, 128 * c0:128 * (c0 + 1)],
                                ae[(tg, st)][:, :],
                                start=(st == 0), stop=(st == 2 * tg + 1))
                    for tg in (ga, gb):
                        if FP8_P7:
                            ct2, iy = c0 // 2, c0 % 2
                            if y[ct2] is None:
                                y[ct2] = projp.tile([128, 2, T], F8,
                                                    tag=f"y{ct2}", name=f"y{ct2}")
                            ysl = y[ct2][:, iy, 256 * tg:256 * (tg + 1)]
                        else:
                            if y[c0] is None:
                                y[c0] = projp.tile([128, T], BF16, tag=f"y{c0}",
                                                   name=f"y{c0}")
                            ysl = y[c0][:, 256 * tg:256 * (tg + 1)]
                        ytmp = misc.tile([128, 256], BF16, tag="ytmp", bufs=3,
                                         name=f"ytmp{tg}_{c0}")
                        nc.vector.tensor_tensor(
                            out=ytmp[:, :], in0=pss[tg][:, 0:256],
                            in1=r_bc[tg // 2][:, 256 * (tg % 2):256 * (tg % 2 + 1)],
                            op=OP.mult)
                        nc.vector.tensor_scalar(
                            out=ysl, in0=ytmp[:, :],
                            scalar1=va[:, c0:c0 + 1], scalar2=vb[:, c0:c0 + 1],
                            op0=OP.mult, op1=OP.add)

            # ------------ P7: single projection via W_eff ----------------
            with tc.tile_pool(name=f"psP{rep}", bufs=2, space="PSUM") as psP:
              for tch in range(TQ):
                for f0 in range(0, 8, 2):
                    pss = [psP.tile([128, 512], F32, tag=f"pp{pi}", bufs=2,
                                    name=f"p2ps{tch}_{f0 + pi}") for pi in range(2)]
                    if FP8_P7:
                        # fp8e4 DoubleRow: each matmul contracts two
                        # 128-channel slices (FD=512 -> ~1.5x regime)
                        for ct2 in range(4):
                            for pi in range(2):
                                nc.tensor.matmul(
                                    pss[pi][:, :],
                                    weff[ct2][:, :, 128 * (f0 + pi):128 * (f0 + pi + 1)],
                                    y[ct2][:, :, 512 * tch:512 * (tch + 1)],
                                    start=(ct2 == 0), stop=(ct2 == 3),
                                    perf_mode=mybir.MatmulPerfMode.DoubleRow)
                    else:
                        for ct in range(8):
                            for pi in range(2):
                                nc.tensor.matmul(
                                    pss[pi][:, :],
                                    weff[ct][:, 128 * (f0 + pi):128 * (f0 + pi + 1)],
                                    y[ct][:, 512 * tch:512 * (tch + 1)],
                                    start=(ct == 0), stop=(ct == 7))
                    for pi in range(2):
                        ft = f0 + pi
                        o_t = outst.tile([128, 512], BF16, tag="o", name=f"o{tch}_{ft}")
                        if FP8_P7:
                            nc.scalar.activation(out=o_t[:, :], in_=pss[pi][:, :],
                                                 func=ACTF.Copy, scale=1.0 / WSCALE)
                        else:
                            nc.scalar.copy(out=o_t[:, :], in_=pss[pi][:, :])
                        nc.sync.dma_start(
                            out=out[128 * ft:128 * (ft + 1), 512 * tch:512 * (tch + 1)],
                            in_=o_t[:, :])


_NC_CACHE = {}


def _get_nc(T):
    if T not in _NC_CACHE:
        _NC_CACHE[T] = build(T)
    return _NC_CACHE[T]


LAST_RESULTS = None
LAST_IN_MAPS = None


def make_in_maps(inputs):
    f = np.float32
    bf = BF16NP
    x = np.asarray(inputs["x"], f)
    B = x.shape[0]
    wT = np.ascontiguousarray(np.asarray(inputs["W_attn"], f).T.astype(bf))  # [C, 3C]
    wp = np.asarray(inputs["W_proj"], f)
    wpT = np.ascontiguousarray(wp.T.astype(bf))                              # [C, C]
    laT = np.ascontiguousarray(np.asarray(inputs["lora_attn_A"], f).T.astype(bf))   # [R, C]
    lbB = np.ascontiguousarray(np.asarray(inputs["lora_attn_B"], f).astype(bf))     # [R, 3C]
    lpaT = np.ascontiguousarray(np.asarray(inputs["lora_proj_A"], f).T.astype(bf))  # [R, C]
    lpbN = np.ascontiguousarray(np.asarray(inputs["lora_proj_B"], f).T.astype(bf))  # [C, R]
    # p-major permutation: gamP[p*24 + i] = gamma[i*128 + p] so on-device
    # readbacks are contiguous per partition
    gam = np.ascontiguousarray(
        np.asarray(inputs["bn_gamma"], f).reshape(24, 128).T.ravel())
    bet = np.ascontiguousarray(
        np.asarray(inputs["bn_beta"], f).reshape(24, 128).T.ravel())
    in_maps = []
    for b in range(B):
        in_maps.append({
            "xT": np.ascontiguousarray(x[b].T.astype(bf)),
            "wT": wT, "wpT": wpT,
            "wpN": np.ascontiguousarray(wp[:, 128 * b:128 * (b + 1)].astype(bf)),
            "laT": laT, "lbB": lbB,
            "lpaT": lpaT, "lpbN": lpbN, "gam": gam, "bet": bet,
        })
    return in_maps


def kernel(x, W_attn, W_proj, lora_attn_A, lora_attn_B, lora_proj_A, lora_proj_B,
           bn_gamma, bn_beta):
    global LAST_RESULTS, LAST_IN_MAPS
    f = np.float32
    x = np.asarray(x, f)
    B, T, C_ = x.shape
    assert C_ == C and B == NCORES

    in_maps = make_in_maps({
        "x": x, "W_attn": W_attn, "W_proj": W_proj,
        "lora_attn_A": lora_attn_A, "lora_attn_B": lora_attn_B,
        "lora_proj_A": lora_proj_A, "lora_proj_B": lora_proj_B,
        "bn_gamma": bn_gamma, "bn_beta": bn_beta})

    LAST_IN_MAPS = in_maps
    nc = _get_nc(T)
    res = run_bass_kernel_spmd(nc, in_maps, core_ids=list(range(NCORES)))
    LAST_RESULTS = res
    return np.stack([np.asarray(res.results[b]["out"]).T for b in range(B)]).astype(f)


# revision 43
# speedup vs baseline: 1.0322x; 1.0032x over previous
"""Trainium2 Bass kernel for nn_Attention_4_lora (B=8, T=1024, C=1024, R=64).

Strategy: data-parallel over the batch dim (1 batch per NeuronCore, 8 cores).
All activations live in transposed [channel, token] layout so that every
matmul contraction runs over the SBUF partition axis. BatchNorm statistics
are reduced across cores with two small AllReduces. All matmul operands are
bf16 (full PE rate, FWL fast-weight-load, half the SBUF/DMA traffic of
f32r); accumulation stays fp32 in PSUM.

Per-core pipeline:
  P1  merge Wm_attn^T = W_attn^T + reshape(A@B)^T on device, in d-HALVES.
      The K=64 LoRA delta matmuls run PAIRED via 2x row tiling (operands
      duplicated on partitions 0-63 / 64-127), so two deltas share the PE
      array. The strided adds alternate between Vector and GpSimd.
  WEFF (between the P2 halves) this core's 128-row shard of
      W_eff^T = Wp^T @ Wmp^T  (Wmp = Wp + lpA@lpB), via
      Z = lpB @ Wp[:, shard]  then  shard = Wp[:, shard]^T Wp^T + Z^T lpA^T
      -- all chained N=512 matmuls; bf16 AllGather of the 8 shards ->
      every core holds the full [C, C] W_eff^T. The readback DMAs are
      issued right after the qk AllReduce so the 2MB (1024-descriptor)
      transfer drains during P3, far away from the P5/P6 seam.
  P2  xa^T[d, t] = Wm^T-slab.T @ x^T for q (half 0) and k (half 1),
      bn_stats per tile
  P3  v[t, c] (natural layout, needed as AV stationary); v stats are
      accumulated across token tiles on GpSimd (sum v) and Vector (sum v^2)
      in f32r, then reduced over the partition axis with 4 ones-matmuls;
      the v AllReduce fires at P3 end.
  P5  scores^T[s, t] in 256-token chunks (finer causal skipping),
      exp((q.k)/32) on ScalarE, causal mask via affine_select, row-sums
      via ones-matmul into half-banks; 1/r via a [128,4] reshape bounce
      (DVE reciprocal is free-dim paced).
  P6  y^T[c, t] = v-slab.T @ att_exp^T; drain fuses 1/r (Vector) and the
      BN-v scale/bias y = y*va[c] + vb[c] (GpSimd), so P7 needs no bias.
  P7  single projection: out^T = W_eff^T-slab.T @ y^T -> [C, T] in bf16.

kernel() takes the full unsharded inputs, shards/uploads (weights cast to
bf16 on host), runs SPMD on cores 0-7, gathers, and transposes back to
[B, T, C].
"""

import ml_dtypes
import numpy as np

import concourse.bass as bass
import concourse.mybir as mybir
import concourse.tile as tile
from concourse import bacc
from concourse.bass_utils import run_bass_kernel_spmd

NCORES = 8
C = 1024
R = 64
D3 = 3 * C
EPS = 1e-5
F32 = mybir.dt.float32
F32R = mybir.dt.float32r
BF16 = mybir.dt.bfloat16
F8 = mybir.dt.float8e4
FP8_P7 = False     # fp8e4 DoubleRow P7 measured 3.6e-2 rel err (> 2e-2 gate)
WSCALE = 1024.0    # W_eff prescale into the fp8e4 normal range
AX = mybir.AxisListType
OP = mybir.AluOpType
ACTF = mybir.ActivationFunctionType
BF16NP = ml_dtypes.bfloat16


def _erange(f, d0, d1):
    """e-range such that d = 3e + f lies in [d0, d1)."""
    el = -((-(d0 - f)) // 3)
    eh = -((-(d1 - f)) // 3)
    return el, eh


def build(T=1024, single_core=False, no_collective=False, reps=1):
    NT = T // 128          # 128-token tiles
    TQ = T // 512          # 512-token chunks
    assert T % 512 == 0

    nc = bacc.Bacc(None, target_bir_lowering=False,
                   num_devices=(1 if single_core else NCORES))

    prm = {}
    prm["xT"] = nc.declare_dram_parameter("xT", [C, T], BF16, isOutput=False)
    prm["wT"] = nc.declare_dram_parameter("wT", [C, D3], BF16, isOutput=False)
    prm["wpT"] = nc.declare_dram_parameter("wpT", [C, C], BF16, isOutput=False)
    prm["wpN"] = nc.declare_dram_parameter("wpN", [C, 128], BF16, isOutput=False)
    prm["laT"] = nc.declare_dram_parameter("laT", [R, C], BF16, isOutput=False)
    prm["lbB"] = nc.declare_dram_parameter("lbB", [R, D3], BF16, isOutput=False)
    prm["lpaT"] = nc.declare_dram_parameter("lpaT", [R, C], BF16, isOutput=False)
    prm["lpbN"] = nc.declare_dram_parameter("lpbN", [C, R], BF16, isOutput=False)
    prm["gam"] = nc.declare_dram_parameter("gam", [D3], F32, isOutput=False)
    prm["bet"] = nc.declare_dram_parameter("bet", [D3], F32, isOutput=False)
    prm["out"] = nc.declare_dram_parameter("out", [C, T], BF16, isOutput=True)

    with tile.TileContext(nc) as tc:
        for rep in range(reps):
            _emit(nc, tc, prm, T, rep, single_core, no_collective)

    nc.compile()
    return nc


def _emit(nc, tc, prm, T, rep, single_core, no_collective):
    NT = T // 128
    TQ = T // 512
    TC4 = T // 256
    xT, wT, wpT, wpN = prm["xT"], prm["wT"], prm["wpT"], prm["wpN"]
    laT, lbB = prm["laT"], prm["lbB"]
    lpaT, lpbN, gam, bet, out = prm["lpaT"], prm["lpbN"], prm["gam"], prm["bet"], prm["out"]

    stats_in = nc.dram_tensor(f"stats_in_{rep}", [4096], F32)
    stats_out = nc.dram_tensor(f"stats_out_{rep}", [4096], F32)
    vstats_in = nc.dram_tensor(f"vstats_in_{rep}", [2 * C], F32)
    vstats_out = nc.dram_tensor(f"vstats_out_{rep}", [2 * C], F32)
    rb_dram = nc.dram_tensor(f"rb_{rep}", [T], F32)
    rb2_dram = nc.dram_tensor(f"rb2_{rep}", [T], F32)
    wdt = F8 if FP8_P7 else BF16
    weff_in = nc.dram_tensor(f"weff_in_{rep}", [128 * C], wdt)
    weff_out = nc.dram_tensor(f"weff_out_{rep}", [C * C], wdt,
                              addr_space="Shared")

    def bcast_dram(param, offset, n):
        return bass.AP(tensor=param[:].tensor, offset=offset, ap=[[0, 128], [1, n]])

    def all_reduce(ins, outs):
        if single_core or no_collective:
            nc.sync.dma_start(out=outs, in_=ins)
        else:
            nc.gpsimd.collective_compute(
                "AllReduce", OP.add,
                replica_groups=[list(range(NCORES))],
                ins=[ins], outs=[outs])

    with (
        tc.tile_pool(name=f"misc{rep}", bufs=1) as misc,
        tc.tile_pool(name=f"outst{rep}", bufs=2) as outst,
        tc.tile_pool(name=f"vpool{rep}", bufs=1) as vpool,
        tc.tile_pool(name=f"attp{rep}", bufs=1) as attp,
        tc.tile_pool(name=f"psA{rep}", bufs=4, space="PSUM") as psA,
    ):
        # ---------------- constants / small loads ----------------
        ones_f = misc.tile([128, 1], F32)
        nc.vector.memset(ones_f[:, :], 1.0)
        ones_b = misc.tile([128, 1], BF16)
        nc.vector.tensor_copy(out=ones_b[:, :], in_=ones_f[:, :])
        ones_r = misc.tile([128, 1], F32R)
        nc.vector.tensor_copy(out=ones_r[:, :], in_=ones_f[:, :])
        eps_t = misc.tile([128, 1], F32)
        nc.vector.memset(eps_t[:, :], EPS)

        qk_mv = misc.tile([128, 16, 2], F32)
        m16 = misc.tile([128, 16], F32)
        qa = misc.tile([128, 16], F32)
        qb = misc.tile([128, 16], F32)

        xa = [None] * 16
        vnat = [None] * NT
        weff = [None] * 8

        with tc.tile_pool(name=f"xapool{rep}", bufs=1) as xapool:
          with tc.tile_pool(name=f"projp{rep}", bufs=1) as projp:
            with tc.tile_pool(name=f"lorap{rep}", bufs=1) as lorap:
                # la/lb duplicated on partitions 0-63 and 64-127 so the K=64
                # delta matmuls can run pairwise via 2x row tiling; the
                # second copy is an SBUF->SBUF DMA on the Scalar DGE queue
                # (no HBM bandwidth, and no blocking of the Sync queue head
                # while it waits for the first copy)
                la2 = lorap.tile([128, C], BF16)
                nc.sync.dma_start(out=la2[0:R, :], in_=laT[:, :])
                nc.scalar.dma_start(out=la2[R:2 * R, :], in_=la2[0:R, :])
                lb2 = lorap.tile([128, D3], BF16)
                for _c in range(3):
                    nc.sync.dma_start(out=lb2[0:R, 1024 * _c:1024 * (_c + 1)],
                                      in_=lbB[:, 1024 * _c:1024 * (_c + 1)])
                    nc.scalar.dma_start(out=lb2[R:2 * R, 1024 * _c:1024 * (_c + 1)],
                                        in_=lb2[0:R, 1024 * _c:1024 * (_c + 1)])


                with tc.tile_pool(name=f"xtpool{rep}", bufs=1) as xtpool:
                    with tc.tile_pool(name=f"wb{rep}", bufs=1) as wbp:
                        # -------- P1 merge (d-halves, paired deltas) ------
                        def merge_half(d0, interleave=None):
                            """Merged Wm^T[:, d0:d0+1024] as 8 c-tiles [128, 1032]."""
                            wq = []
                            for ct in range(8):
                                w_t = wbp.tile([128, 1032], BF16, tag=f"wb{ct}",
                                               bufs=(2 if ct < 4 else 1),
                                               name=f"wh{d0}_{ct}")
                                nc.sync.dma_start(
                                    out=w_t[:, 0:1024],
                                    in_=wT[128 * ct:128 * (ct + 1), d0:d0 + 1024])
                                if interleave is not None:
                                    interleave(ct)
                                wq.append(w_t)
                            # f-major so the first delta pairs only need the
                            # first lb chunk to have landed
                            jobs = [(ct, f) for f in range(3) for ct in range(8)]
                            for j0 in range(0, len(jobs), 2):
                                pss = []
                                for pi, (ct, f) in enumerate(jobs[j0:j0 + 2]):
                                    r0 = R * pi      # partition offset 0 / 64
                                    el, eh = _erange(f, d0, d0 + 1024)
                                    cnt = eh - el
                                    ps = psA.tile([128, 512], F32, tag="mm",
                                                  name=f"dps{d0}_{ct}_{f}")
                                    nc.tensor.matmul(
                                        ps[:, 0:cnt],
                                        lb2[r0:r0 + R,
                                            1024 * f + 128 * ct:1024 * f + 128 * (ct + 1)],
                                        la2[r0:r0 + R, el:el + cnt],
                                        start=True, stop=True)
                                    pss.append(ps)
                                for pi, (ct, f) in enumerate(jobs[j0:j0 + 2]):
                                    w_t = wq[ct]
                                    view3 = w_t[:, :].rearrange(
                                        "p (u three) -> p u three", three=3)
                                    el, eh = _erange(f, d0, d0 + 1024)
                                    cnt = eh - el
                                    c0 = 3 * el + f - d0
                                    ps = pss[pi]
                                    if (ct + f) % 2 == 0:
                                        nc.vector.tensor_tensor(
                                            out=view3[:, 0:cnt, c0],
                                            in0=view3[:, 0:cnt, c0],
                                            in1=ps[:, 0:cnt], op=OP.add)
                                    else:
                                        # GpSimd cannot read PSUM: stage via
                                        # Scalar, add on GpSimd
                                        stg = misc.tile([128, 344], BF16,
                                                        tag="mgst", bufs=2,
                                                        name=f"mgst{d0}_{ct}_{f}")
                                        nc.scalar.copy(out=stg[:, 0:cnt],
                                                       in_=ps[:, 0:cnt])
                                        nc.gpsimd.tensor_tensor(
                                            out=view3[:, 0:cnt, c0],
                                            in0=view3[:, 0:cnt, c0],
                                            in1=stg[:, 0:cnt], op=OP.add)
                            return wq

                        xt = [None] * 8

                        def load_xt(k):
                            x_t = xtpool.tile([128, T], BF16, tag=f"xt{k}",
                                              name=f"xt{k}")
                            nc.sync.dma_start(out=x_t[:, :],
                                              in_=xT[128 * k:128 * (k + 1), :])
                            xt[k] = x_t

                        # interleave wT0/xt loads so p2's k-th chain step
                        # gets both of its operands at the same time
                        wq0 = merge_half(0, interleave=load_xt)

                        def p2_half(H, wq):
                            for il in range(8):
                                g = 8 * H + il
                                xa_g = xapool.tile([128, T], BF16, tag=f"xa{g}",
                                                   name=f"xa{g}")
                                # two interleaved chains (one per 512-token
                                # chunk) hide the per-chain ldweights bubbles
                                pss = [psA.tile([128, 512], F32, tag="mm",
                                                name=f"xaps{g}_{tch}")
                                       for tch in range(TQ)]
                                for k in range(8):
                                    for tch in range(TQ):
                                        nc.tensor.matmul(
                                            pss[tch][:, :],
                                            wq[k][:, 128 * il:128 * (il + 1)],
                                            xt[k][:, 512 * tch:512 * (tch + 1)],
                                            start=(k == 0), stop=(k == 7))
                                for tch in range(TQ):
                                    nc.scalar.copy(out=xa_g[:, 512 * tch:512 * (tch + 1)],
                                                   in_=pss[tch][:, :])
                                bnstat = misc.tile([128, TQ, 6], F32, tag="bnstat",
                                                   bufs=2, name=f"bnstat{g}")
                                for j in range(TQ):
                                    nc.vector.bn_stats(out=bnstat[:, j, :],
                                                       in_=xa_g[:, 512 * j:512 * (j + 1)])
                                nc.vector.bn_aggr(out=qk_mv[:, g, :], in_=bnstat[:, :, :])
                                xa[g] = xa_g

                        p2_half(0, wq0)          # q channels d in [0, 1024)

                        # -------- WEFF: this core's 128-col shard of
                        # W_eff^T = Wp^T @ Wmp^T with Wmp = Wp + lpA@lpB
                        with tc.tile_pool(name=f"wefc{rep}", bufs=1) as wefc:
                            lpa_sb = wefc.tile([R, C], BF16)
                            nc.sync.dma_start(out=lpa_sb[:, :], in_=lpaT[:, :])
                            lpbn = wefc.tile([128, 8 * R], BF16)
                            for et in range(8):
                                nc.sync.dma_start(
                                    out=lpbn[:, R * et:R * (et + 1)],
                                    in_=lpbN[128 * et:128 * (et + 1), :])
                            wpn = wefc.tile([128, C], BF16)
                            for et in range(8):
                                nc.sync.dma_start(
                                    out=wpn[:, 128 * et:128 * (et + 1)],
                                    in_=wpN[128 * et:128 * (et + 1), :])
                            z_sb = wefc.tile([R, 128], BF16)
                            ps = psA.tile([128, 512], F32, tag="mm", name="zps")
                            for et in range(8):
                                nc.tensor.matmul(
                                    ps[0:R, 0:128],
                                    lpbn[:, R * et:R * (et + 1)],
                                    wpn[:, 128 * et:128 * (et + 1)],
                                    start=(et == 0), stop=(et == 7))
                            nc.scalar.copy(out=z_sb[:, :], in_=ps[0:R, 0:128])
                            # full wpT rows [128, 1024] per e-tile feed both
                            # fc chains from one DMA
                            w2f = []
                            for et in range(8):
                                w2 = wefc.tile([128, C], BF16, tag=f"wpt{et}",
                                               bufs=1, name=f"wpt{et}")
                                nc.sync.dma_start(
                                    out=w2[:, :],
                                    in_=wpT[128 * et:128 * (et + 1), :])
                                w2f.append(w2)
                            psw = [psA.tile([128, 512], F32, tag="mm",
                                            name=f"weffps{fc}") for fc in range(2)]
                            for et in range(8):
                                for fc in range(2):
                                    nc.tensor.matmul(
                                        psw[fc][:, :],
                                        wpn[:, 128 * et:128 * (et + 1)],
                                        w2f[et][:, 512 * fc:512 * (fc + 1)],
                                        start=(et == 0), stop=False)
                            for fc in range(2):
                                nc.tensor.matmul(
                                    psw[fc][:, :], z_sb[:, :],
                                    lpa_sb[:, 512 * fc:512 * (fc + 1)],
                                    start=False, stop=True)
                                wst = wefc.tile([128, 512], wdt, tag="wst", bufs=2,
                                                name=f"weffst{fc}")
                                if FP8_P7:
                                    # prescale into fp8e4 normal range; the
                                    # P7 drain divides it back out
                                    nc.scalar.activation(
                                        out=wst[:, :], in_=psw[fc][:, :],
                                        func=ACTF.Copy, scale=WSCALE)
                                else:
                                    nc.vector.tensor_copy(out=wst[:, :],
                                                          in_=psw[fc][:, :])
                                nc.sync.dma_start(
                                    out=bass.AP(tensor=weff_in[:].tensor,
                                                offset=512 * fc,
                                                ap=[[C, 128], [1, 512]]),
                                    in_=wst[:, :])
                        if single_core or no_collective:
                            # local fallback: replicate shard into all 8 slots
                            for ct in range(8):
                                nc.sync.dma_start(
                                    out=weff_out[128 * C * ct:128 * C * (ct + 1)],
                                    in_=weff_in[:])
                        else:
                            nc.gpsimd.collective_compute(
                                "AllGather", OP.bypass,
                                replica_groups=[list(range(NCORES))],
                                ins=[weff_in[:]], outs=[weff_out[:]])

                        wq1 = merge_half(1024)
                        p2_half(1, wq1)          # k channels d in [1024, 2048)

                        # qk stats -> (mean, E[x^2]) packed, DMA to stats_in
                        nc.vector.tensor_tensor(out=m16[:, :], in0=qk_mv[:, :, 0],
                                                in1=qk_mv[:, :, 0], op=OP.mult)
                        nc.vector.tensor_tensor(out=qk_mv[:, :, 1], in0=qk_mv[:, :, 1],
                                                in1=m16[:, :], op=OP.add)
                        nc.sync.dma_start(
                            out=stats_in[0:4096].rearrange("(p i s) -> p i s", p=128, s=2),
                            in_=qk_mv[:, :, :])
                        all_reduce(stats_in[:], stats_out[:])

                        # W_eff readback: issue now so it drains during P3,
                        # far from the P5/P6 seam.
                        if FP8_P7:
                            # DoubleRow layout [c-part, 2, f]: channel
                            # c = 256*ct2 + 128*i + p
                            for ct2 in range(4):
                                weff[ct2] = projp.tile([128, 2, C], F8,
                                                       tag=f"wf{ct2}",
                                                       name=f"wf{ct2}")
                                nc.sync.dma_start(
                                    out=weff[ct2][:, :, :],
                                    in_=weff_out[256 * C * ct2:256 * C * (ct2 + 1)]
                                    .rearrange("(i p f) -> p i f", i=2, p=128))
                        else:
                            for ct in range(8):
                                weff[ct] = projp.tile([128, C], BF16, tag=f"wf{ct}",
                                                      name=f"wf{ct}")
                                nc.sync.dma_start(
                                    out=weff[ct][:, :],
                                    in_=weff_out[128 * C * ct:128 * C * (ct + 1)]
                                    .rearrange("(p i) -> p i", p=128))
                        # gam/bet are uploaded p-major (gamP[p*24+i] =
                        # gamma[i*128+p]) so every readback is contiguous
                        # per partition instead of a 4-byte-descriptor bomb
                        gv8 = misc.tile([128, 8], F32)
                        nc.sync.dma_start(
                            out=gv8[:, :],
                            in_=bass.AP(tensor=gam[:].tensor, offset=16,
                                        ap=[[24, 128], [1, 8]]))
                        bv8 = misc.tile([128, 8], F32)
                        nc.sync.dma_start(
                            out=bv8[:, :],
                            in_=bass.AP(tensor=bet[:].tensor, offset=16,
                                        ap=[[24, 128], [1, 8]]))

                        # ---------------- P3: v natural + stats ----------------
                        with tc.tile_pool(name=f"psV{rep}", bufs=1, space="PSUM") as psV:
                            wqv = merge_half(2048)

                            # qk-stats readback + normalize: all Vector-engine
                            # so nothing fences the Scalar P3 drain queue;
                            # runs as soon as the AllReduce lands.
                            gqk = misc.tile([128, 16], F32)
                            nc.sync.dma_start(
                                out=gqk[:, :],
                                in_=bass.AP(tensor=gam[:].tensor, offset=0,
                                            ap=[[24, 128], [1, 16]]))
                            bqk = misc.tile([128, 16], F32)
                            nc.sync.dma_start(
                                out=bqk[:, :],
                                in_=bass.AP(tensor=bet[:].tensor, offset=0,
                                            ap=[[24, 128], [1, 16]]))
                            ar_qk = misc.tile([128, 16, 2], F32)
                            nc.sync.dma_start(
                                out=ar_qk[:, :, :],
                                in_=stats_out[0:4096].rearrange("(p i s) -> p i s", p=128, s=2))
                            # q,k: a = gamma*rstd, b = beta - mean*a
                            nc.vector.tensor_scalar(out=ar_qk[:, :, 0], in0=ar_qk[:, :, 0],
                                                    scalar1=1.0 / NCORES, scalar2=None, op0=OP.mult)
                            nc.vector.tensor_scalar(out=ar_qk[:, :, 1], in0=ar_qk[:, :, 1],
                                                    scalar1=1.0 / NCORES, scalar2=None, op0=OP.mult)
                            nc.vector.tensor_tensor(out=m16[:, :], in0=ar_qk[:, :, 0],
                                                    in1=ar_qk[:, :, 0], op=OP.mult)
                            nc.vector.tensor_tensor(out=m16[:, :], in0=ar_qk[:, :, 1],
                                                    in1=m16[:, :], op=OP.subtract)

                            def finish_norm():
                                # emitted mid-P3 so the Scalar queue reaches the
                                # Sqrt only after the AllReduce has landed
                                nc.scalar.activation(out=m16[:, :], in_=m16[:, :],
                                                     func=ACTF.Sqrt, bias=eps_t[:, 0:1])
                                nc.vector.reciprocal(out=m16[:, :], in_=m16[:, :])
                                nc.vector.tensor_tensor(out=qa[:, :], in0=m16[:, :],
                                                        in1=gqk[:, :], op=OP.mult)
                                nc.vector.tensor_tensor(out=qb[:, :], in0=ar_qk[:, :, 0],
                                                        in1=qa[:, :], op=OP.mult)
                                nc.vector.tensor_tensor(out=qb[:, :], in0=bqk[:, :],
                                                        in1=qb[:, :], op=OP.subtract)
                                for g in range(16):
                                    nc.vector.tensor_scalar(
                                        out=xa[g][:, :], in0=xa[g][:, :],
                                        scalar1=qa[:, g:g + 1], scalar2=qb[:, g:g + 1],
                                        op0=OP.mult, op1=OP.add)

                            # v stats: accumulate sum(v) on GpSimd and
                            # sum(v^2) on Vector across token tiles (f32r),
                            # reduce over partitions with 4 ones-matmuls at
                            # the end -> v AllReduce fires at P3 end.
                            acc_v = [None, None]
                            acc_q = [None, None]
                            for tt in range(NT):
                                vnat[tt] = vpool.tile([128, C], BF16,
                                                      tag=f"v{tt}", name=f"v{tt}")
                                pss = [psA.tile([128, 512], F32, tag="mm",
                                                name=f"vps{hc}_{tt}")
                                       for hc in range(2)]
                                for k in range(8):
                                    for hc in range(2):
                                        nc.tensor.matmul(
                                            pss[hc][:, :],
                                            xt[k][:, 128 * tt:128 * (tt + 1)],
                                            wqv[k][:, 512 * hc:512 * (hc + 1)],
                                            start=(k == 0), stop=(k == 7))
                                for hc in range(2):
                                    vsl = vnat[tt][:, 512 * hc:512 * (hc + 1)]
                                    nc.scalar.copy(out=vsl, in_=pss[hc][:, :])
                                    sq = misc.tile([128, 512], BF16, tag="sq", bufs=3,
                                                   name=f"sq{hc}_{tt}")
                                    nc.scalar.activation(
                                        out=sq[:, :], in_=pss[hc][:, :], func=ACTF.Square)
                                    if tt == 0:
                                        acc_v[hc] = misc.tile([128, 512], F32R,
                                                              tag=f"accv{hc}", bufs=1,
                                                              name=f"accv{hc}")
                                        acc_q[hc] = misc.tile([128, 512], F32R,
                                                              tag=f"accq{hc}", bufs=1,
                                                              name=f"accq{hc}")
                                        nc.vector.tensor_copy(out=acc_v[hc][:, :], in_=vsl)
                                        nc.vector.tensor_copy(out=acc_q[hc][:, :], in_=sq[:, :])
                                    else:
                                        nc.vector.tensor_tensor(
                                            out=acc_v[hc][:, :], in0=acc_v[hc][:, :],
                                            in1=vsl, op=OP.add)
                                        nc.vector.tensor_tensor(
                                            out=acc_q[hc][:, :], in0=acc_q[hc][:, :],
                                            in1=sq[:, :], op=OP.add)
                            # after ALL P3 drains so the AllReduce-gated
                            # Sqrt can never stall the Scalar drain queue
                            finish_norm()

                            for hc in range(2):
                                ps_v = psV.tile([1, 512], F32, tag=f"fv{hc}",
                                                name=f"psfv{hc}")
                                nc.tensor.matmul(ps_v[0:1, :], ones_r[:, :],
                                                 acc_v[hc][:, :], start=True, stop=True)
                                vst1 = misc.tile([1, 512], F32, tag="vst", bufs=4,
                                                 name=f"vst1_{hc}")
                                nc.scalar.copy(out=vst1[0:1, :], in_=ps_v[0:1, :])
                                nc.sync.dma_start(
                                    out=vstats_in[512 * hc:512 * (hc + 1)],
                                    in_=vst1[0:1, :])
                                ps_q = psV.tile([1, 512], F32, tag=f"fq{hc}",
                                                name=f"psfq{hc}")
                                nc.tensor.matmul(ps_q[0:1, :], ones_r[:, :],
                                                 acc_q[hc][:, :], start=True, stop=True)
                                vst2 = misc.tile([1, 512], F32, tag="vst", bufs=4,
                                                 name=f"vst2_{hc}")
                                nc.scalar.copy(out=vst2[0:1, :], in_=ps_q[0:1, :])
                                nc.sync.dma_start(
                                    out=vstats_in[C + 512 * hc:C + 512 * (hc + 1)],
                                    in_=vst2[0:1, :])
                            all_reduce(vstats_in[:], vstats_out[:])
                            # v-stats readback emitted here so its many tiny
                            # descriptors drain mid-P5 (right after the
                            # AllReduce lands), not at the P5/P6 seam
                            vs_m = misc.tile([128, 8], F32)
                            nc.sync.dma_start(
                                out=vs_m[:, :],
                                in_=vstats_out[0:C].rearrange("(i p) -> p i", p=128))
                            vs_e = misc.tile([128, 8], F32)
                            nc.sync.dma_start(
                                out=vs_e[:, :],
                                in_=vstats_out[C:2 * C].rearrange("(i p) -> p i", p=128))

            # lorap/xtpool/wbp closed; their SBUF is free for P5 tiles.
            # two independent 1/r broadcast tiles so the early P6 drains
            # never wait on the second broadcast DMA
            r_bc = [projp.tile([128, T // 2], F32, tag=f"rbc{pr}", name=f"rbc{pr}")
                    for pr in range(2)]
            with tc.tile_pool(name=f"bc{rep}", bufs=1) as bcp:
                    rstage = bcp.tile([128, T], F32)   # row 0: r staging
                    # ------------ P5: scores^T, exp, causal, row sums ----
                    # 256-token chunks: finer causal skipping than 512.
                    ae = {}
                    scale = 1.0 / float(np.sqrt(C))
                    with tc.tile_pool(name=f"psR{rep}", bufs=1, space="PSUM") as psR:
                        items = []
                        for tc4 in range(TC4):
                            acts = [st for st in range(NT) if 128 * st < 256 * (tc4 + 1)]
                            for ii, st in enumerate(acts):
                                items.append((tc4, st, ii, len(acts)))
                        # two chunks share one [1,512] bank (half each);
                        # safe: chains are sequential in PE order and the
                        # drain happens after both halves' chains finish
                        ps_rs = {pr: psR.tile([1, 512], F32, tag=f"r{pr}",
                                              name=f"psr{pr}") for pr in range(TC4 // 2)}

                        def r_chain(pr):
                            # both halves of ps_rs[pr] are complete: 1/r via
                            # a [128,4] reshape bounce (DVE reciprocal is
                            # free-dim paced, so [1,512] would cost 3.3us).
                            # All DMAs ride the Scalar engine's HW-DGE queue
                            # (idle at P5 end), decoupled from the Sync
                            # queue's completion-slot chains.
                            nc.scalar.copy(
                                out=rstage[0:1, 512 * pr:512 * (pr + 1)],
                                in_=ps_rs[pr][0:1, :])
                            nc.scalar.dma_start(
                                out=rb_dram[512 * pr:512 * (pr + 1)],
                                in_=rstage[0:1, 512 * pr:512 * (pr + 1)])
                            # p-major contiguous mapping (order is irrelevant
                            # for a pointwise reciprocal): 64B descriptors
                            # instead of a 512x4B scatter
                            rp = bcp.tile([32, 16], F32, tag="rp",
                                          bufs=2, name=f"rp{pr}")
                            nc.scalar.dma_start(
                                out=rp[:, :],
                                in_=rb_dram[512 * pr:512 * (pr + 1)].rearrange(
                                    "(p i) -> p i", p=32))
                            nc.vector.reciprocal(out=rp[:, :], in_=rp[:, :])
                            nc.scalar.dma_start(
                                out=rb2_dram[512 * pr:512 * (pr + 1)].rearrange(
                                    "(p i) -> p i", p=32),
                                in_=rp[:, :])
                            nc.scalar.dma_start(
                                out=r_bc[pr][:, :],
                                in_=bcast_dram(rb2_dram, 512 * pr, 512))

                        for p0 in range(0, len(items), 2):
                            pair = items[p0:p0 + 2]
                            pss = []
                            for (tc4, st, ii, na) in pair:
                                pss.append(psA.tile([128, 512], F32, tag="mm",
                                                    name=f"scps{tc4}_{st}"))
                            for j in range(8):
                                for pi, (tc4, st, ii, na) in enumerate(pair):
                                    nc.tensor.matmul(
                                        pss[pi][:, 0:256],
                                        xa[8 + j][:, 128 * st:128 * (st + 1)],
                                        xa[j][:, 256 * tc4:256 * (tc4 + 1)],
                                        start=(j == 0), stop=(j == 7))
                            for pi, (tc4, st, ii, na) in enumerate(pair):
                                a_t = attp.tile([128, 256], BF16, tag=f"ae{tc4}_{st}",
                                                name=f"ae{tc4}_{st}")
                                nc.scalar.activation(out=a_t[:, :], in_=pss[pi][:, 0:256],
                                                     func=ACTF.Exp, scale=scale)
                                base = 256 * tc4 - 128 * st
                                if base < 127:
                                    nc.gpsimd.affine_select(
                                        out=a_t[:, :], in_=a_t[:, :],
                                        pattern=[[1, 256]], base=base,
                                        channel_multiplier=-1,
                                        compare_op=OP.is_ge, fill=0.0)
                                pr, half = tc4 // 2, tc4 % 2
                                nc.tensor.matmul(
                                    ps_rs[pr][0:1, 256 * half:256 * (half + 1)],
                                    ones_b[:, :], a_t[:, :],
                                    start=(ii == 0), stop=(ii == na - 1))
                                ae[(tc4, st)] = a_t
                                if ii == na - 1 and half == 1:
                                    r_chain(pr)

            # -------- v scale/bias math (stats already read back) ----
            m8 = misc.tile([128, 8], F32)
            va = misc.tile([128, 8], F32)
            vb = misc.tile([128, 8], F32)
            inv_n = 1.0 / ((1 if single_core else NCORES) * T)

            def vavb_math():
                nc.vector.tensor_scalar(out=vs_m[:, :], in0=vs_m[:, :],
                                        scalar1=inv_n, scalar2=None, op0=OP.mult)
                nc.vector.tensor_scalar(out=vs_e[:, :], in0=vs_e[:, :],
                                        scalar1=inv_n, scalar2=None, op0=OP.mult)
                nc.vector.tensor_tensor(out=m8[:, :], in0=vs_m[:, :], in1=vs_m[:, :], op=OP.mult)
                nc.vector.tensor_tensor(out=m8[:, :], in0=vs_e[:, :], in1=m8[:, :], op=OP.subtract)
                nc.scalar.activation(out=m8[:, :], in_=m8[:, :], func=ACTF.Sqrt,
                                     bias=eps_t[:, 0:1])
                nc.vector.reciprocal(out=m8[:, :], in_=m8[:, :])
                nc.vector.tensor_tensor(out=va[:, :], in0=m8[:, :], in1=gv8[:, :], op=OP.mult)
                nc.vector.tensor_tensor(out=vb[:, :], in0=vs_m[:, :], in1=va[:, :], op=OP.mult)
                nc.vector.tensor_tensor(out=vb[:, :], in0=bv8[:, :], in1=vb[:, :], op=OP.subtract)

            # ---------------- P6: AV + fused BN-v on the drain ---------
            # y = (AV * 1/r) * va[c] + vb[c]. Chains pair two tc4 chunks at
            # the SAME (st, c0) so consecutive matmuls share the stationary
            # operand -- the N=256 ldweights then amortizes over two
            # matmuls instead of being exposed every matmul.
            vavb_math()
            y = [None] * 8
            for ga, gb in ((0, 1), (2, 3)):
                acts_a = list(range(2 * ga + 2))
                acts_b = list(range(2 * gb + 2))
                for c0 in range(8):
                    pss = {tg: psA.tile([128, 512], F32, tag="mm",
                                        name=f"avps{tg}_{c0}") for tg in (ga, gb)}
                    for st in acts_b:
                        for tg in (ga, gb):
                            if st > 2 * tg + 1:
                                continue
                            nc.tensor.matmul(
                                pss[tg][:, 0:256],
                                vnat[st][:, 128 * c0:128 * (c0 + 1)],
                                ae[(tg, st)][:, :],
                                start=(st == 0), stop=(st == 2 * tg + 1))
                    for tg in (ga, gb):
                        if FP8_P7:
                            ct2, iy = c0 // 2, c0 % 2
                            if y[ct2] is None:
                                y[ct2] = projp.tile([128, 2, T], F8,
                                                    tag=f"y{ct2}", name=f"y{ct2}")
                            ysl = y[ct2][:, iy, 256 * tg:256 * (tg + 1)]
                        else:
                            if y[c0] is None:
                                y[c0] = projp.tile([128, T], BF16, tag=f"y{c0}",
                                                   name=f"y{c0}")
                            ysl = y[c0][:, 256 * tg:256 * (tg + 1)]
                        ytmp = misc.tile([128, 256], BF16, tag="ytmp", bufs=3,
                                         name=f"ytmp{tg}_{c0}")
                        nc.vector.tensor_tensor(
                            out=ytmp[:, :], in0=pss[tg][:, 0:256],
                            in1=r_bc[tg // 2][:, 256 * (tg % 2):256 * (tg % 2 + 1)],
                            op=OP.mult)
                        nc.vector.tensor_scalar(
                            out=ysl, in0=ytmp[:, :],
                            scalar1=va[:, c0:c0 + 1], scalar2=vb[:, c0:c0 + 1],
                            op0=OP.mult, op1=OP.add)

            # ------------ P7: single projection via W_eff ----------------
            with tc.tile_pool(name=f"psP{rep}", bufs=2, space="PSUM") as psP:
              for tch in range(TQ):
                for f0 in range(0, 8, 2):
                    pss = [psP.tile([128, 512], F32, tag=f"pp{pi}", bufs=2,
                                    name=f"p2ps{tch}_{f0 + pi}") for pi in range(2)]
                    if FP8_P7:
                        # fp8e4 DoubleRow: each matmul contracts two
                        # 128-channel slices (FD=512 -> ~1.5x regime)
                        for ct2 in range(4):
                            for pi in range(2):
                                nc.tensor.matmul(
                                    pss[pi][:, :],
                                    weff[ct2][:, :, 128 * (f0 + pi):128 * (f0 + pi + 1)],
                                    y[ct2][:, :, 512 * tch:512 * (tch + 1)],
                                    start=(ct2 == 0), stop=(ct2 == 3),
                                    perf_mode=mybir.MatmulPerfMode.DoubleRow)
                    else:
                        for ct in range(8):
                            for pi in range(2):
                                nc.tensor.matmul(
                                    pss[pi][:, :],
                                    weff[ct][:, 128 * (f0 + pi):128 * (f0 + pi + 1)],
                                    y[ct][:, 512 * tch:512 * (tch + 1)],
                                    start=(ct == 0), stop=(ct == 7))
                    for pi in range(2):
                        ft = f0 + pi
                        o_t = outst.tile([128, 512], BF16, tag="o", name=f"o{tch}_{ft}")
                        if FP8_P7:
                            nc.scalar.activation(out=o_t[:, :], in_=pss[pi][:, :],
                                                 func=ACTF.Copy, scale=1.0 / WSCALE)
                        else:
                            nc.scalar.copy(out=o_t[:, :], in_=pss[pi][:, :])
                        nc.sync.dma_start(
                            out=out[128 * ft:128 * (ft + 1), 512 * tch:512 * (tch + 1)],
                            in_=o_t[:, :])


_NC_CACHE = {}


def _get_nc(T):
    if T not in _NC_CACHE:
        _NC_CACHE[T] = build(T)
    return _NC_CACHE[T]


LAST_RESULTS = None
LAST_IN_MAPS = None


def make_in_maps(inputs):
    f = np.float32
    bf = BF16NP
    x = np.asarray(inputs["x"], f)
    B = x.shape[0]
    wT = np.ascontiguousarray(np.asarray(inputs["W_attn"], f).T.astype(bf))  # [C, 3C]
    wp = np.asarray(inputs["W_proj"], f)
    wpT = np.ascontiguousarray(wp.T.astype(bf))                              # [C, C]
    laT = np.ascontiguousarray(np.asarray(inputs["lora_attn_A"], f).T.astype(bf))   # [R, C]
    lbB = np.ascontiguousarray(np.asarray(inputs["lora_attn_B"], f).astype(bf))     # [R, 3C]
    lpaT = np.ascontiguousarray(np.asarray(inputs["lora_proj_A"], f).T.astype(bf))  # [R, C]
    lpbN = np.ascontiguousarray(np.asarray(inputs["lora_proj_B"], f).T.astype(bf))  # [C, R]
    # p-major permutation: gamP[p*24 + i] = gamma[i*128 + p] so on-device
    # readbacks are contiguous per partition
    gam = np.ascontiguousarray(
        np.asarray(inputs["bn_gamma"], f).reshape(24, 128).T.ravel())
    bet = np.ascontiguousarray(
        np.asarray(inputs["bn_beta"], f).reshape(24, 128).T.ravel())
    in_maps = []
    for b in range(B):
        in_maps.append({
            "xT": np.ascontiguousarray(x[b].T.astype(bf)),
            "wT": wT, "wpT": wpT,
            "wpN": np.ascontiguousarray(wp[:, 128 * b:128 * (b + 1)].astype(bf)),
            "laT": laT, "lbB": lbB,
            "lpaT": lpaT, "lpbN": lpbN, "gam": gam, "bet": bet,
        })
    return in_maps


def kernel(x, W_attn, W_proj, lora_attn_A, lora_attn_B, lora_proj_A, lora_proj_B,
           bn_gamma, bn_beta):
    global LAST_RESULTS, LAST_IN_MAPS
    f = np.float32
    x = np.asarray(x, f)
    B, T, C_ = x.shape
    assert C_ == C and B == NCORES

    in_maps = make_in_maps({
        "x": x, "W_attn": W_attn, "W_proj": W_proj,
        "lora_attn_A": lora_attn_A, "lora_attn_B": lora_attn_B,
        "lora_proj_A": lora_proj_A, "lora_proj_B": lora_proj_B,
        "bn_gamma": bn_gamma, "bn_beta": bn_beta})

    LAST_IN_MAPS = in_maps
    nc = _get_nc(T)
    res = run_bass_kernel_spmd(nc, in_maps, core_ids=list(range(NCORES)))
    LAST_RESULTS = res
    return np.stack([np.asarray(res.results[b]["out"]).T for b in range(B)]).astype(f)
